# revision 3
# baseline (speedup 1.0000x reference)
"""Multi-head attention (B=2, S=2048, D=1024, H=16) on 8 TRN2 NeuronCores.

Sharding: data-parallel over batch (2 groups of 4 cores), tensor-parallel over
heads within a group (4 heads = 256 feature columns per core). Each core:
  - projects its batch's q/k/v (full D contraction) into its 256-col head slice
  - runs full attention for its 4 heads over the 2048-token sequence
  - applies its 256-row slice of w_o, producing a partial [D, S] output (bf16)
Host sums the 4 partials per batch (+ b_o folded into one core per batch) and
transposes back to [S, D].

V1 structure (vs the 236us baseline):
  - attention for (sb0, jt0) is emitted INSIDE the per-tb projection loop, so
    the ScalarE exp conveyor starts at ~11us instead of ~43us
  - all four q-block projections happen in the startup phase (PE slack under
    pair-0's ACT-bound stretch); boundaries only carry out-proj groups
  - exp runs at N=2048 per ACTIVATE where possible: PSUM phase 2 is
    scA [128,2048] (4 banks) + scB [128,1024] (2 banks) alternating, cutting
    the 352-cycle per-call ACT overhead; phase 1 (startup) uses its own
    psProj + sc0 pools that are closed before phase 2 opens (8-bank budget)
  - out-proj psum tiles ride the scB/scA rotation at pair boundaries
  - `out` is bf16 (halves output DMA traffic; host sums partials in fp32)
Softmax denominator comes from a ones-column appended to each head's V tile
(PSUM-accumulated by the P@V matmul). Scores matmuls are auto row-tiled
(64-partition lhsT at base 0/64 -> PE quadrant concurrency).
"""

import numpy as np

B, S, D, H = 2, 2048, 1024, 16
DK = D // H          # 64
NCORES = 8
GROUPS = 4           # head-groups (cores) per batch
JC = D // GROUPS     # 256 feature columns per core (4 heads)
TB = 512             # token block (matmul moving free dim)
NTB = S // TB        # 4
NDT = D // 128       # 8 contraction tiles for projections
NTT = S // 128       # 16 key-token tiles per sequence
VROW = 2 * (DK + 1)  # 130: per-jt vp row segment (2 heads x (64 v cols + ones))

_NC = None


def _build():
    import concourse.mybir as mybir
    import concourse.tile as tile
    from concourse import bacc
    from concourse.masks import make_identity

    f32 = mybir.dt.float32
    bf16 = mybir.dt.bfloat16
    AF = mybir.ActivationFunctionType

    nc = bacc.Bacc("TRN2", target_bir_lowering=False, debug=False, num_devices=NCORES)

    qT = nc.dram_tensor("qT", [D, S], bf16, kind="ExternalInput").ap()
    kT = nc.dram_tensor("kT", [D, S], bf16, kind="ExternalInput").ap()
    vT = nc.dram_tensor("vT", [D, S], bf16, kind="ExternalInput").ap()
    wq = nc.dram_tensor("wq", [D, JC], bf16, kind="ExternalInput").ap()
    wk = nc.dram_tensor("wk", [D, JC], bf16, kind="ExternalInput").ap()
    wv = nc.dram_tensor("wv", [D, JC], bf16, kind="ExternalInput").ap()
    wo = nc.dram_tensor("wo", [JC, D], bf16, kind="ExternalInput").ap()
    bq = nc.dram_tensor("bq", [128, 2], f32, kind="ExternalInput").ap()
    bk = nc.dram_tensor("bk", [128, 2], f32, kind="ExternalInput").ap()
    bv = nc.dram_tensor("bv", [128, 2], f32, kind="ExternalInput").ap()
    bo = nc.dram_tensor("bo", [128, 8], f32, kind="ExternalInput").ap()
    out = nc.dram_tensor("out", [D, S], bf16, kind="ExternalOutput").ap()

    with tile.TileContext(nc) as tc:
        with (
            tc.tile_pool(name="const", bufs=1) as const,
            tc.tile_pool(name="inp", bufs=5) as inpool,
            tc.tile_pool(name="expp", bufs=4) as exppool,
            tc.tile_pool(name="usb", bufs=4) as usbpool,
            tc.tile_pool(name="nrm", bufs=4) as nrmpool,
            tc.tile_pool(name="osb", bufs=2) as osbpool,
            tc.tile_pool(name="psU", bufs=2, space="PSUM") as psU,
        ):
            # ---- constants / weights (k first: it gates the first projection) ----
            wkv = const.tile([128, 2 * NDT * JC], bf16, tag="wkv")
            nc.sync.dma_start(
                wkv[:, 0:NDT * JC].rearrange("p (dt j) -> p dt j", dt=NDT),
                wk[:].rearrange("(dt p) j -> p dt j", p=128),
            )
            nc.sync.dma_start(
                wkv[:, NDT * JC:].rearrange("p (dt j) -> p dt j", dt=NDT),
                wv[:].rearrange("(dt p) j -> p dt j", p=128),
            )
            wk_sb = [wkv[:, d * JC:(d + 1) * JC] for d in range(NDT)]
            wv_sb = [wkv[:, NDT * JC + d * JC: NDT * JC + (d + 1) * JC] for d in range(NDT)]

            def load_w(ap_dram, name, n_dt):
                cols = ap_dram.shape[1]
                t = const.tile([128, n_dt * cols], bf16, tag=name)
                nc.sync.dma_start(
                    t[:].rearrange("p (dt j) -> p dt j", dt=n_dt),
                    ap_dram[:].rearrange("(dt p) j -> p dt j", p=128),
                )
                return [t[:, d * cols:(d + 1) * cols] for d in range(n_dt)]

            def load_b(ap_dram, name, cols):
                t = const.tile([128, cols], f32, tag=name)
                nc.sync.dma_start(t[:], ap_dram[:])
                return t

            wq_sb = load_w(wq, "wq", NDT)
            bq_sb = load_b(bq, "bq", 2)
            bk_sb = load_b(bk, "bk", 2)
            bv_sb = load_b(bv, "bv", 2)
            bo_sb = load_b(bo, "bo", 8)

            ident = const.tile([128, 128], bf16, tag="ident")
            make_identity(nc, ident[:])

            # ---- persistent activations (feature-major) ----
            qpT = const.tile([128, 2 * S], bf16, tag="qpT")
            kpT = const.tile([128, 2 * S], bf16, tag="kpT")
            vpT = const.tile([128, 2 * S], bf16, tag="vpT")
            vp = const.tile([128, NTT * 2 * VROW], bf16, tag="vp")
            hoT = const.tile([128, 2 * S], bf16, tag="hoT")

            ones_src = const.tile([128, 1], f32, tag="ones_src")
            nc.gpsimd.memset(ones_src[:], 1.0)
            vp_ones = vp[:].rearrange(
                "p (tt seg c) -> p (tt seg) c", tt=NTT, seg=4, c=DK + 1
            )[:, :, DK:DK + 1]
            nc.vector.tensor_copy(vp_ones, ones_src[:].to_broadcast([128, NTT * 4, 1]))

            # ---- projections ----
            def proj_tb(xT_dram, w_tiles, b_tile, dstT, tb, pool, split=1):
                xt = inpool.tile([128, NDT * TB], bf16, tag="in")
                hd = NDT // split
                for h in range(split):
                    nc.sync.dma_start(
                        xt[:, h * hd * TB:(h + 1) * hd * TB].rearrange(
                            "p (dt t) -> p dt t", dt=hd),
                        xT_dram[h * hd * 128:(h + 1) * hd * 128,
                                tb * TB:(tb + 1) * TB].rearrange(
                            "(dt p) t -> p dt t", p=128),
                    )
                xtiles = [xt[:, d * TB:(d + 1) * TB] for d in range(NDT)]
                for jt in range(2):
                    ps = pool.tile([128, TB], f32, tag="mm")
                    for d in range(NDT):
                        nc.tensor.matmul(
                            ps[:],
                            lhsT=w_tiles[d][:, jt * 128:(jt + 1) * 128],
                            rhs=xtiles[d],
                            start=(d == 0),
                            stop=(d == NDT - 1),
                        )
                    nc.vector.tensor_scalar_add(
                        dstT[:, jt * S + tb * TB: jt * S + (tb + 1) * TB],
                        ps[:],
                        b_tile[:, jt:jt + 1],
                    )

            def vp_tb(tb, pool):
                for tt in range(tb * 4, (tb + 1) * 4):
                    for jt in range(2):
                        tp = pool.tile([128, 128], bf16, tag="mm")
                        nc.tensor.transpose(
                            tp[:], vpT[:, jt * S + tt * 128: jt * S + (tt + 1) * 128],
                            ident[:],
                        )
                        o = tt * 2 * VROW + jt * VROW
                        nc.vector.tensor_copy(vp[:, o: o + DK], tp[:, 0:DK])
                        nc.vector.tensor_copy(
                            vp[:, o + DK + 1: o + 2 * DK + 1], tp[:, DK:2 * DK])

            # ---- attention pieces ----
            def turn(sb, jt, tts, uA, uB, pool):
                """Scores + exp + P@V for a group of token tiles.

                Each tt's scores are two auto-row-tiled matmuls (lhsT at
                partition 0/64 -> PE quadrants run them concurrently); one
                ACTIVATE covers the whole group (N = len(tts)*1024).
                """
                n = len(tts)
                sc = pool.tile([128, n * 2 * TB], f32, tag="sc")
                for ti, tt in enumerate(tts):
                    for h, p0 in ((0, 0), (1, 64)):
                        nc.tensor.matmul(
                            sc[:, (2 * ti + h) * TB:(2 * ti + h + 1) * TB],
                            lhsT=kpT[p0:p0 + DK, jt * S + tt * 128: jt * S + (tt + 1) * 128],
                            rhs=qpT[p0:p0 + DK, jt * S + sb * TB: jt * S + (sb + 1) * TB],
                        )
                ex = exppool.tile([128, n * 2 * TB], bf16, tag="exp")
                nc.scalar.activation(ex[:], sc[:], AF.Exp, scale=float(1.0 / np.sqrt(DK)))
                for ti, tt in enumerate(tts):
                    for h, u in ((0, uA), (1, uB)):
                        o = tt * 2 * VROW + jt * VROW + h * (DK + 1)
                        nc.tensor.matmul(
                            u[:],
                            lhsT=vp[:, o: o + DK + 1],
                            rhs=ex[:, (2 * ti + h) * TB:(2 * ti + h + 1) * TB],
                            start=(tt == 0),
                            stop=(tt == NTT - 1),
                        )

            def norm_pair(sb, jt, uA, uB):
                for h, u in ((0, uA), (1, uB)):
                    usb = usbpool.tile([DK + 1, TB], f32, tag="usb")
                    nc.vector.tensor_copy(usb[:], u[:])
                    rc = nrmpool.tile([1, TB], f32, tag="rc")
                    nc.sync.dma_start(rc[:], usb[DK:DK + 1, :])
                    rc2 = nrmpool.tile([1, TB], f32, tag="rc2")
                    nc.vector.reciprocal_approx_fast(rc2[:], rc[:])
                    rb = nrmpool.tile([DK, TB], f32, tag="rb")
                    nc.gpsimd.partition_broadcast(rb[:], rc2[:])
                    if h == 0:
                        nc.vector.tensor_mul(
                            hoT[0:DK, jt * S + sb * TB: jt * S + (sb + 1) * TB],
                            usb[0:DK, :],
                            rb[:],
                        )
                    else:
                        tmp = nrmpool.tile([DK, TB], bf16, tag="tmp")
                        nc.vector.tensor_mul(tmp[:], usb[0:DK, :], rb[:])
                        nc.sync.dma_start(
                            hoT[DK:2 * DK, jt * S + sb * TB: jt * S + (sb + 1) * TB],
                            tmp[:],
                        )

            # out-proj for query block sb, emitted as 4 [128,1024] psum tiles
            # (2 feature tiles each) interleaved into a pair's sc rotation
            def outproj_group(sb, g, ot, pool, spread_evac=False):
                op = pool.tile([128, 2 * TB], f32, tag="sc")
                for fi in range(2):
                    ft = 2 * g + fi
                    for jt in range(2):
                        nc.tensor.matmul(
                            op[:, fi * TB:(fi + 1) * TB],
                            lhsT=wo_sb[jt][:, ft * 128:(ft + 1) * 128],
                            rhs=hoT[:, jt * S + sb * TB: jt * S + (sb + 1) * TB],
                            start=(jt == 0),
                            stop=(jt == 1),
                        )
                    if spread_evac and fi == 1:
                        nc.scalar.activation(
                            ot[:, ft * TB:(ft + 1) * TB], op[:, fi * TB:(fi + 1) * TB],
                            AF.Identity, bias=bo_sb[:, ft:ft + 1],
                        )
                    else:
                        nc.vector.tensor_scalar_add(
                            ot[:, ft * TB:(ft + 1) * TB], op[:, fi * TB:(fi + 1) * TB],
                            bo_sb[:, ft:ft + 1],
                        )
                if g == 1 or g == 3:
                    h0 = 0 if g == 1 else 512
                    nc.sync.dma_start(
                        out[h0:h0 + 512, sb * TB:(sb + 1) * TB].rearrange(
                            "(ft p) t -> p ft t", p=128),
                        ot[:, (g - 1) * 2 * TB:(g + 1) * 2 * TB].rearrange(
                            "p (ft t) -> p ft t", ft=4),
                    )

            # ================= phase 1: startup =================
            # kv (+all q) projections, interleaved with attention pair (0,0)
            TURN_TTS = [[0, 1], [2], [3, 4], [5], [6, 7], [8],
                        [9, 10], [11], [12, 13], [14], [15]]

            with (
                tc.tile_pool(name="psProj", bufs=2, space="PSUM") as psProj,
                tc.tile_pool(name="psSC0", bufs=2, space="PSUM") as psSC0,
            ):
                uA = psU.tile([DK + 1, TB], f32, tag="U")
                uB = psU.tile([DK + 1, TB], f32, tag="U")
                for tb in range(NTB):
                    proj_tb(kT, wk_sb, bk_sb, kpT, tb, psProj, split=2 if tb == 0 else 1)
                    proj_tb(vT, wv_sb, bv_sb, vpT, tb, psProj)
                    vp_tb(tb, psProj)
                    if tb == 0:
                        proj_tb(qT, wq_sb, bq_sb, qpT, 0, psProj)
                    # emit pair (0, 0) turns whose keys are now projected
                    # (single-tt turns: psSC0 slots are [128, 1024])
                    for tt in range(4 * tb, 4 * tb + 4):
                        turn(0, 0, [tt], uA, uB, psSC0)
                # q blocks 1-3: fill PE slack under pair (0,0)'s ACT stretch
                for sb in range(1, NTB):
                    proj_tb(qT, wq_sb, bq_sb, qpT, sb, psProj)
                wo_sb = load_w(wo, "wo", 2)
                norm_pair(0, 0, uA, uB)

            # ================= phase 2: attention conveyor =================
            with (
                tc.tile_pool(name="psSCA", bufs=1, space="PSUM") as psSCA,
                tc.tile_pool(name="psSCB", bufs=1, space="PSUM") as psSCB,
            ):
                # force slot sizes: scA 4 banks, scB 2 banks
                pairs = [(0, 1)] + [(sb, jt) for sb in range(1, NTB) for jt in range(2)]
                for sb, jt in pairs:
                    uA = psU.tile([DK + 1, TB], f32, tag="U")
                    uB = psU.tile([DK + 1, TB], f32, tag="U")
                    # interleave out-proj of sb-1 into this pair's B-slot turns
                    do_op = (jt == 0 and sb > 0)
                    if do_op:
                        ot = osbpool.tile([128, 8 * TB], bf16, tag="ot")
                    og = 0
                    for tts in TURN_TTS:
                        pool = psSCA if len(tts) == 2 else psSCB
                        turn(sb, jt, tts, uA, uB, pool)
                        if do_op and len(tts) == 1 and og < 4:
                            outproj_group(sb - 1, og, ot, psSCB)
                            og += 1
                    norm_pair(sb, jt, uA, uB)
                # final out-proj (tail: spread evac across scalar+vector)
                ot = osbpool.tile([128, 8 * TB], bf16, tag="ot")
                for g in range(4):
                    outproj_group(NTB - 1, g, ot, psSCB, spread_evac=True)

    nc.compile()
    return nc


def _get_nc():
    global _NC
    if _NC is None:
        _NC = _build()
    return _NC


def make_in_maps(q, k, v, w_q, b_q, w_k, b_k, w_v, b_v, w_o, b_o):
    import ml_dtypes
    cdt = ml_dtypes.bfloat16
    q = np.asarray(q, np.float32)
    k = np.asarray(k, np.float32)
    v = np.asarray(v, np.float32)
    w_q = np.asarray(w_q, np.float32)
    w_k = np.asarray(w_k, np.float32)
    w_v = np.asarray(w_v, np.float32)
    w_o = np.asarray(w_o, np.float32)
    b_q = np.asarray(b_q, np.float32)
    b_k = np.asarray(b_k, np.float32)
    b_v = np.asarray(b_v, np.float32)
    b_o = np.asarray(b_o, np.float32)

    in_maps = []
    for c in range(NCORES):
        b, g = divmod(c, GROUPS)
        js = slice(g * JC, (g + 1) * JC)
        bias2 = lambda x: np.ascontiguousarray(x[js].reshape(2, 128).T)
        in_maps.append({
            "qT": np.ascontiguousarray(q[b].T).astype(cdt),
            "kT": np.ascontiguousarray(k[b].T).astype(cdt),
            "vT": np.ascontiguousarray(v[b].T).astype(cdt),
            "wq": np.ascontiguousarray(w_q[:, js]).astype(cdt),
            "wk": np.ascontiguousarray(w_k[:, js]).astype(cdt),
            "wv": np.ascontiguousarray(w_v[:, js]).astype(cdt),
            "wo": np.ascontiguousarray(w_o[js, :]).astype(cdt),
            "bq": bias2(b_q),
            "bk": bias2(b_k),
            "bv": bias2(b_v),
            "bo": np.ascontiguousarray(b_o.reshape(8, 128).T)
            if g == 0 else np.zeros((128, 8), np.float32),
        })
    return in_maps


def gather(results):
    out = np.zeros((B, S, D), np.float32)
    for c in range(NCORES):
        b = c // GROUPS
        out[b] += results[c]["out"].T.astype(np.float32)
    return out


def kernel(q, k, v, w_q, b_q, w_k, b_k, w_v, b_v, w_o, b_o, _trace=False):
    from concourse.bass_utils import run_bass_kernel_spmd

    nc = _get_nc()
    in_maps = make_in_maps(q, k, v, w_q, b_q, w_k, b_k, w_v, b_v, w_o, b_o)
    res = run_bass_kernel_spmd(nc, in_maps, core_ids=list(range(NCORES)), trace=_trace)
    out = gather(res.results)
    if _trace:
        kernel.last_exec_time_ns = res.exec_time_ns
        kernel.last_results = res
    return out


# revision 6
# speedup vs baseline: 1.2305x; 1.2305x over previous
"""Multi-head attention (B=2, S=2048, D=1024, H=16) on 8 TRN2 NeuronCores.

Sharding: data-parallel over batch (2 groups of 4 cores), tensor-parallel over
heads within a group (4 heads = 256 feature columns per core). Each core:
  - projects its batch's q/k/v (full D contraction) into its 256-col head slice
  - runs full attention for its 4 heads over the 2048-token sequence
  - applies its 256-row slice of w_o, producing a partial [D, S] output (bf16)
Host sums the 4 partials per batch (+ b_o folded into one core per batch) and
transposes back to [S, D].

V1 structure (vs the 236us baseline):
  - attention for (sb0, jt0) is emitted INSIDE the per-tb projection loop, so
    the ScalarE exp conveyor starts at ~11us instead of ~43us
  - all four q-block projections happen in the startup phase (PE slack under
    pair-0's ACT-bound stretch); boundaries only carry out-proj groups
  - exp runs at N=2048 per ACTIVATE where possible: PSUM phase 2 is
    scA [128,2048] (4 banks) + scB [128,1024] (2 banks) alternating, cutting
    the 352-cycle per-call ACT overhead; phase 1 (startup) uses its own
    psProj + sc0 pools that are closed before phase 2 opens (8-bank budget)
  - out-proj psum tiles ride the scB/scA rotation at pair boundaries
  - `out` is bf16 (halves output DMA traffic; host sums partials in fp32)
Softmax denominator comes from a ones-column appended to each head's V tile
(PSUM-accumulated by the P@V matmul). Scores matmuls are auto row-tiled
(64-partition lhsT at base 0/64 -> PE quadrant concurrency).
"""

import numpy as np

B, S, D, H = 2, 2048, 1024, 16
DK = D // H          # 64
NCORES = 8
GROUPS = 4           # head-groups (cores) per batch
JC = D // GROUPS     # 256 feature columns per core (4 heads)
TB = 512             # token block (matmul moving free dim)
NTB = S // TB        # 4
NDT = D // 128       # 8 contraction tiles for projections
NTT = S // 128       # 16 key-token tiles per sequence
VROW = 2 * (DK + 1)  # 130: per-jt vp row segment (2 heads x (64 v cols + ones))

_NC = None


def _build():
    import concourse.mybir as mybir
    import concourse.tile as tile
    from concourse import bacc
    from concourse.masks import make_identity

    f32 = mybir.dt.float32
    bf16 = mybir.dt.bfloat16
    AF = mybir.ActivationFunctionType

    nc = bacc.Bacc("TRN2", target_bir_lowering=False, debug=False, num_devices=NCORES)

    qT = nc.dram_tensor("qT", [D, S], bf16, kind="ExternalInput").ap()
    kT = nc.dram_tensor("kT", [D, S], bf16, kind="ExternalInput").ap()
    vT = nc.dram_tensor("vT", [D, S], bf16, kind="ExternalInput").ap()
    wq = nc.dram_tensor("wq", [D, JC], bf16, kind="ExternalInput").ap()
    wk = nc.dram_tensor("wk", [D, JC], bf16, kind="ExternalInput").ap()
    wv = nc.dram_tensor("wv", [D, JC], bf16, kind="ExternalInput").ap()
    wo = nc.dram_tensor("wo", [JC, D], bf16, kind="ExternalInput").ap()
    bq = nc.dram_tensor("bq", [128, 2], f32, kind="ExternalInput").ap()
    bk = nc.dram_tensor("bk", [128, 2], f32, kind="ExternalInput").ap()
    bv = nc.dram_tensor("bv", [128, 2], f32, kind="ExternalInput").ap()
    bo = nc.dram_tensor("bo", [128, 8], f32, kind="ExternalInput").ap()
    out = nc.dram_tensor("out", [D, S], bf16, kind="ExternalOutput").ap()

    with tile.TileContext(nc) as tc:
        with (
            tc.tile_pool(name="const", bufs=1) as const,
            tc.tile_pool(name="inp", bufs=5) as inpool,
            tc.tile_pool(name="expp", bufs=4) as exppool,
            tc.tile_pool(name="usb", bufs=4) as usbpool,
            tc.tile_pool(name="nrm", bufs=4) as nrmpool,
            tc.tile_pool(name="osb", bufs=2) as osbpool,
            tc.tile_pool(name="psU", bufs=2, space="PSUM") as psU,
        ):
            # ---- constants / weights (k first: it gates the first projection) ----
            wkv = const.tile([128, 2 * NDT * JC], bf16, tag="wkv")
            nc.sync.dma_start(
                wkv[:, 0:NDT * JC].rearrange("p (dt j) -> p dt j", dt=NDT),
                wk[:].rearrange("(dt p) j -> p dt j", p=128),
            )
            nc.sync.dma_start(
                wkv[:, NDT * JC:].rearrange("p (dt j) -> p dt j", dt=NDT),
                wv[:].rearrange("(dt p) j -> p dt j", p=128),
            )
            wk_sb = [wkv[:, d * JC:(d + 1) * JC] for d in range(NDT)]
            wv_sb = [wkv[:, NDT * JC + d * JC: NDT * JC + (d + 1) * JC] for d in range(NDT)]

            def load_w(ap_dram, name, n_dt):
                cols = ap_dram.shape[1]
                t = const.tile([128, n_dt * cols], bf16, tag=name)
                nc.sync.dma_start(
                    t[:].rearrange("p (dt j) -> p dt j", dt=n_dt),
                    ap_dram[:].rearrange("(dt p) j -> p dt j", p=128),
                )
                return [t[:, d * cols:(d + 1) * cols] for d in range(n_dt)]

            def load_b(ap_dram, name, cols):
                t = const.tile([128, cols], f32, tag=name)
                nc.sync.dma_start(t[:], ap_dram[:])
                return t

            wq_sb = load_w(wq, "wq", NDT)
            bq_sb = load_b(bq, "bq", 2)
            bk_sb = load_b(bk, "bk", 2)
            bv_sb = load_b(bv, "bv", 2)
            bo_sb = load_b(bo, "bo", 8)

            ident = const.tile([128, 128], bf16, tag="ident")
            make_identity(nc, ident[:])

            # ---- persistent activations (feature-major) ----
            qpT = const.tile([128, 2 * S], bf16, tag="qpT")
            kpT = const.tile([128, 2 * S], bf16, tag="kpT")
            vpT = const.tile([128, 2 * S], bf16, tag="vpT")
            vp = const.tile([128, NTT * 2 * VROW], bf16, tag="vp")
            hoT = const.tile([128, 2 * S], bf16, tag="hoT")

            ones_src = const.tile([128, 1], f32, tag="ones_src")
            nc.gpsimd.memset(ones_src[:], 1.0)
            vp_ones = vp[:].rearrange(
                "p (tt seg c) -> p (tt seg) c", tt=NTT, seg=4, c=DK + 1
            )[:, :, DK:DK + 1]
            nc.vector.tensor_copy(vp_ones, ones_src[:].to_broadcast([128, NTT * 4, 1]))

            # ---- projections ----
            def proj_tb(xT_dram, w_tiles, b_tile, dstT, tb, pool, split=1):
                xt = inpool.tile([128, NDT * TB], bf16, tag="in")
                hd = NDT // split
                for h in range(split):
                    nc.sync.dma_start(
                        xt[:, h * hd * TB:(h + 1) * hd * TB].rearrange(
                            "p (dt t) -> p dt t", dt=hd),
                        xT_dram[h * hd * 128:(h + 1) * hd * 128,
                                tb * TB:(tb + 1) * TB].rearrange(
                            "(dt p) t -> p dt t", p=128),
                    )
                xtiles = [xt[:, d * TB:(d + 1) * TB] for d in range(NDT)]
                for jt in range(2):
                    ps = pool.tile([128, TB], f32, tag="mm")
                    for d in range(NDT):
                        nc.tensor.matmul(
                            ps[:],
                            lhsT=w_tiles[d][:, jt * 128:(jt + 1) * 128],
                            rhs=xtiles[d],
                            start=(d == 0),
                            stop=(d == NDT - 1),
                        )
                    nc.vector.tensor_scalar_add(
                        dstT[:, jt * S + tb * TB: jt * S + (tb + 1) * TB],
                        ps[:],
                        b_tile[:, jt:jt + 1],
                    )

            def vp_tb(tb, pool):
                for tt in range(tb * 4, (tb + 1) * 4):
                    for jt in range(2):
                        tp = pool.tile([128, 128], bf16, tag="mm")
                        nc.tensor.transpose(
                            tp[:], vpT[:, jt * S + tt * 128: jt * S + (tt + 1) * 128],
                            ident[:],
                        )
                        o = tt * 2 * VROW + jt * VROW
                        nc.vector.tensor_copy(vp[:, o: o + DK], tp[:, 0:DK])
                        nc.vector.tensor_copy(
                            vp[:, o + DK + 1: o + 2 * DK + 1], tp[:, DK:2 * DK])

            # ---- attention pieces ----
            # The PE queue executes in order: a P@V matmul waiting on its exp
            # stalls everything emitted after it. So scores and P@V are split
            # and emitted as a 1-deep software pipeline (next turn's scores
            # always go in front of the previous turn's P@V).
            def turn_scores(sb, jt, tts, pool):
                n = len(tts)
                sc = pool.tile([128, n * 2 * TB], f32, tag="sc")
                for ti, tt in enumerate(tts):
                    for h, p0 in ((0, 0), (1, 64)):
                        nc.tensor.matmul(
                            sc[:, (2 * ti + h) * TB:(2 * ti + h + 1) * TB],
                            lhsT=kpT[p0:p0 + DK, jt * S + tt * 128: jt * S + (tt + 1) * 128],
                            rhs=qpT[p0:p0 + DK, jt * S + sb * TB: jt * S + (sb + 1) * TB],
                        )
                ex = exppool.tile([128, n * 2 * TB], bf16, tag="exp")
                nc.scalar.activation(ex[:], sc[:], AF.Exp, scale=float(1.0 / np.sqrt(DK)))
                return (tts, ex)

            def turn_pv(sb, jt, pend, uA, uB):
                tts, ex = pend
                for ti, tt in enumerate(tts):
                    for h, u in ((0, uA), (1, uB)):
                        o = tt * 2 * VROW + jt * VROW + h * (DK + 1)
                        nc.tensor.matmul(
                            u[:],
                            lhsT=vp[:, o: o + DK + 1],
                            rhs=ex[:, (2 * ti + h) * TB:(2 * ti + h + 1) * TB],
                            start=(tt == 0),
                            stop=(tt == NTT - 1),
                        )

            def norm_pair(sb, jt, uA, uB):
                for h, u in ((0, uA), (1, uB)):
                    usb = usbpool.tile([DK + 1, TB], f32, tag="usb")
                    nc.vector.tensor_copy(usb[:], u[:])
                    rc = nrmpool.tile([1, TB], f32, tag="rc")
                    nc.sync.dma_start(rc[:], usb[DK:DK + 1, :])
                    rc2 = nrmpool.tile([1, TB], f32, tag="rc2")
                    nc.vector.reciprocal_approx_fast(rc2[:], rc[:])
                    rb = nrmpool.tile([DK, TB], f32, tag="rb")
                    nc.gpsimd.partition_broadcast(rb[:], rc2[:])
                    if h == 0:
                        nc.vector.tensor_mul(
                            hoT[0:DK, jt * S + sb * TB: jt * S + (sb + 1) * TB],
                            usb[0:DK, :],
                            rb[:],
                        )
                    else:
                        tmp = nrmpool.tile([DK, TB], bf16, tag="tmp")
                        nc.vector.tensor_mul(tmp[:], usb[0:DK, :], rb[:])
                        nc.sync.dma_start(
                            hoT[DK:2 * DK, jt * S + sb * TB: jt * S + (sb + 1) * TB],
                            tmp[:],
                        )

            # out-proj for query block sb, emitted as 4 [128,1024] psum tiles
            # (2 feature tiles each) interleaved into a pair's sc rotation
            def outproj_group(sb, g, ot, pool, spread_evac=False):
                op = pool.tile([128, 2 * TB], f32, tag="sc")
                for fi in range(2):
                    ft = 2 * g + fi
                    for jt in range(2):
                        nc.tensor.matmul(
                            op[:, fi * TB:(fi + 1) * TB],
                            lhsT=wo_sb[jt][:, ft * 128:(ft + 1) * 128],
                            rhs=hoT[:, jt * S + sb * TB: jt * S + (sb + 1) * TB],
                            start=(jt == 0),
                            stop=(jt == 1),
                        )
                    if spread_evac and fi == 1:
                        nc.scalar.activation(
                            ot[:, ft * TB:(ft + 1) * TB], op[:, fi * TB:(fi + 1) * TB],
                            AF.Identity, bias=bo_sb[:, ft:ft + 1],
                        )
                    else:
                        nc.vector.tensor_scalar_add(
                            ot[:, ft * TB:(ft + 1) * TB], op[:, fi * TB:(fi + 1) * TB],
                            bo_sb[:, ft:ft + 1],
                        )
                if g == 1 or g == 3:
                    h0 = 0 if g == 1 else 512
                    nc.sync.dma_start(
                        out[h0:h0 + 512, sb * TB:(sb + 1) * TB].rearrange(
                            "(ft p) t -> p ft t", p=128),
                        ot[:, (g - 1) * 2 * TB:(g + 1) * 2 * TB].rearrange(
                            "p (ft t) -> p ft t", ft=4),
                    )

            # ================= phase 1: startup =================
            # kv (+all q) projections, interleaved with attention pair (0,0)
            TURN_TTS = [[0, 1], [2], [3, 4], [5], [6, 7], [8],
                        [9, 10], [11], [12, 13], [14], [15]]

            with (
                tc.tile_pool(name="psProj", bufs=2, space="PSUM") as psProj,
                tc.tile_pool(name="psSC0", bufs=2, space="PSUM") as psSC0,
            ):
                uA = psU.tile([DK + 1, TB], f32, tag="U")
                uB = psU.tile([DK + 1, TB], f32, tag="U")
                pend = None
                for tb in range(NTB):
                    proj_tb(kT, wk_sb, bk_sb, kpT, tb, psProj, split=2 if tb == 0 else 1)
                    proj_tb(vT, wv_sb, bv_sb, vpT, tb, psProj)
                    vp_tb(tb, psProj)
                    if tb == 0:
                        proj_tb(qT, wq_sb, bq_sb, qpT, 0, psProj)
                    # emit pair (0, 0) turns whose keys are now projected
                    # (single-tt turns: psSC0 slots are [128, 1024])
                    for tt in range(4 * tb, 4 * tb + 4):
                        nxt = turn_scores(0, 0, [tt], psSC0)
                        if pend is not None:
                            turn_pv(0, 0, pend, uA, uB)
                        pend = nxt
                # q blocks 1-3: fill PE slack under pair (0,0)'s ACT stretch
                for sb in range(1, NTB):
                    proj_tb(qT, wq_sb, bq_sb, qpT, sb, psProj)
                wo_sb = load_w(wo, "wo", 2)
                turn_pv(0, 0, pend, uA, uB)
                norm_pair(0, 0, uA, uB)

            # ================= phase 2: attention conveyor =================
            with (
                tc.tile_pool(name="psSCA", bufs=1, space="PSUM") as psSCA,
                tc.tile_pool(name="psSCB", bufs=1, space="PSUM") as psSCB,
            ):
                pairs = [(0, 1)] + [(sb, jt) for sb in range(1, NTB) for jt in range(2)]
                for sb, jt in pairs:
                    uA = psU.tile([DK + 1, TB], f32, tag="U")
                    uB = psU.tile([DK + 1, TB], f32, tag="U")
                    # interleave out-proj of sb-1 into this pair's B-slot turns
                    do_op = (jt == 0 and sb > 0)
                    if do_op:
                        ot = osbpool.tile([128, 8 * TB], bf16, tag="ot")
                    og = 0
                    pend = None
                    for tts in TURN_TTS:
                        pool = psSCA if len(tts) == 2 else psSCB
                        nxt = turn_scores(sb, jt, tts, pool)
                        if pend is not None:
                            turn_pv(sb, jt, pend, uA, uB)
                        pend = nxt
                        if do_op and len(tts) == 1 and og < 4:
                            outproj_group(sb - 1, og, ot, psSCB)
                            og += 1
                    turn_pv(sb, jt, pend, uA, uB)
                    norm_pair(sb, jt, uA, uB)
                # final out-proj (tail: spread evac across scalar+vector)
                ot = osbpool.tile([128, 8 * TB], bf16, tag="ot")
                for g in range(4):
                    outproj_group(NTB - 1, g, ot, psSCB, spread_evac=True)

    nc.compile()
    return nc


def _get_nc():
    global _NC
    if _NC is None:
        _NC = _build()
    return _NC


def make_in_maps(q, k, v, w_q, b_q, w_k, b_k, w_v, b_v, w_o, b_o):
    import ml_dtypes
    cdt = ml_dtypes.bfloat16
    q = np.asarray(q, np.float32)
    k = np.asarray(k, np.float32)
    v = np.asarray(v, np.float32)
    w_q = np.asarray(w_q, np.float32)
    w_k = np.asarray(w_k, np.float32)
    w_v = np.asarray(w_v, np.float32)
    w_o = np.asarray(w_o, np.float32)
    b_q = np.asarray(b_q, np.float32)
    b_k = np.asarray(b_k, np.float32)
    b_v = np.asarray(b_v, np.float32)
    b_o = np.asarray(b_o, np.float32)

    in_maps = []
    for c in range(NCORES):
        b, g = divmod(c, GROUPS)
        js = slice(g * JC, (g + 1) * JC)
        bias2 = lambda x: np.ascontiguousarray(x[js].reshape(2, 128).T)
        in_maps.append({
            "qT": np.ascontiguousarray(q[b].T).astype(cdt),
            "kT": np.ascontiguousarray(k[b].T).astype(cdt),
            "vT": np.ascontiguousarray(v[b].T).astype(cdt),
            "wq": np.ascontiguousarray(w_q[:, js]).astype(cdt),
            "wk": np.ascontiguousarray(w_k[:, js]).astype(cdt),
            "wv": np.ascontiguousarray(w_v[:, js]).astype(cdt),
            "wo": np.ascontiguousarray(w_o[js, :]).astype(cdt),
            "bq": bias2(b_q),
            "bk": bias2(b_k),
            "bv": bias2(b_v),
            "bo": np.ascontiguousarray(b_o.reshape(8, 128).T)
            if g == 0 else np.zeros((128, 8), np.float32),
        })
    return in_maps


def gather(results):
    out = np.zeros((B, S, D), np.float32)
    for c in range(NCORES):
        b = c // GROUPS
        out[b] += results[c]["out"].T.astype(np.float32)
    return out


def kernel(q, k, v, w_q, b_q, w_k, b_k, w_v, b_v, w_o, b_o, _trace=False):
    from concourse.bass_utils import run_bass_kernel_spmd

    nc = _get_nc()
    in_maps = make_in_maps(q, k, v, w_q, b_q, w_k, b_k, w_v, b_v, w_o, b_o)
    res = run_bass_kernel_spmd(nc, in_maps, core_ids=list(range(NCORES)), trace=_trace)
    out = gather(res.results)
    if _trace:
        kernel.last_exec_time_ns = res.exec_time_ns
        kernel.last_results = res
    return out


# revision 12
# speedup vs baseline: 1.4122x; 1.1477x over previous
"""Multi-head attention (B=2, S=2048, D=1024, H=16) on 8 TRN2 NeuronCores.

Sharding: data-parallel over batch (2 groups of 4 cores), tensor-parallel over
heads within a group (4 heads = 256 feature columns per core). Each core:
  - projects its batch's q/k/v (full D contraction) into its 256-col head slice
  - runs full attention for its 4 heads over the 2048-token sequence
  - applies its 256-row slice of w_o, producing a partial [D, S] output (bf16)
Host sums the 4 partials per batch (+ b_o folded into one core per batch) and
transposes back to [S, D].

V1 structure (vs the 236us baseline):
  - attention for (sb0, jt0) is emitted INSIDE the per-tb projection loop, so
    the ScalarE exp conveyor starts at ~11us instead of ~43us
  - all four q-block projections happen in the startup phase (PE slack under
    pair-0's ACT-bound stretch); boundaries only carry out-proj groups
  - exp runs at N=2048 per ACTIVATE where possible: PSUM phase 2 is
    scA [128,2048] (4 banks) + scB [128,1024] (2 banks) alternating, cutting
    the 352-cycle per-call ACT overhead; phase 1 (startup) uses its own
    psProj + sc0 pools that are closed before phase 2 opens (8-bank budget)
  - out-proj psum tiles ride the scB/scA rotation at pair boundaries
  - `out` is bf16 (halves output DMA traffic; host sums partials in fp32)
Softmax denominator comes from a ones-column appended to each head's V tile
(PSUM-accumulated by the P@V matmul). Scores matmuls are auto row-tiled
(64-partition lhsT at base 0/64 -> PE quadrant concurrency).
"""

import numpy as np

B, S, D, H = 2, 2048, 1024, 16
DK = D // H          # 64
NCORES = 8
GROUPS = 4           # head-groups (cores) per batch
JC = D // GROUPS     # 256 feature columns per core (4 heads)
TB = 512             # token block (matmul moving free dim)
NTB = S // TB        # 4
NDT = D // 128       # 8 contraction tiles for projections
NTT = S // 128       # 16 key-token tiles per sequence
VROW = 2 * (DK + 1)  # 130: per-jt vp row segment (2 heads x (64 v cols + ones))

_NC = None


def _build():
    import concourse.mybir as mybir
    import concourse.tile as tile
    from concourse import bacc
    from concourse.masks import make_identity

    f32 = mybir.dt.float32
    bf16 = mybir.dt.bfloat16
    AF = mybir.ActivationFunctionType

    nc = bacc.Bacc("TRN2", target_bir_lowering=False, debug=False, num_devices=NCORES)

    qT = nc.dram_tensor("qT", [D, S], bf16, kind="ExternalInput").ap()
    kT = nc.dram_tensor("kT", [D, S], bf16, kind="ExternalInput").ap()
    vT = nc.dram_tensor("vT", [D, S], bf16, kind="ExternalInput").ap()
    wq = nc.dram_tensor("wq", [D, JC], bf16, kind="ExternalInput").ap()
    wk = nc.dram_tensor("wk", [D, JC], bf16, kind="ExternalInput").ap()
    wv = nc.dram_tensor("wv", [D, JC], bf16, kind="ExternalInput").ap()
    wo = nc.dram_tensor("wo", [JC, D], bf16, kind="ExternalInput").ap()
    # all biases in one DMA: cols 0-1 bq, 2-3 bk, 4-5 bv, 6-13 bo
    bias = nc.dram_tensor("bias", [128, 14], f32, kind="ExternalInput").ap()
    out = nc.dram_tensor("out", [D, S], bf16, kind="ExternalOutput").ap()

    with tile.TileContext(nc) as tc:
        with (
            tc.tile_pool(name="const", bufs=1) as const,
            tc.tile_pool(name="inp", bufs=5) as inpool,
            tc.tile_pool(name="expp", bufs=4) as exppool,
            tc.tile_pool(name="usb", bufs=4) as usbpool,
            tc.tile_pool(name="nrm", bufs=4) as nrmpool,
            tc.tile_pool(name="osb", bufs=2) as osbpool,
            tc.tile_pool(name="psU", bufs=2, space="PSUM") as psU,
        ):
            # ---- weights (DMA emission order = critical path order:
            #      wk, bias, [kT0], wq, [qT0], wv, [vT0], ...) ----
            def load_w(ap_dram, name, n_dt):
                cols = ap_dram.shape[1]
                t = const.tile([128, n_dt * cols], bf16, tag=name)
                nc.sync.dma_start(
                    t[:].rearrange("p (dt j) -> p dt j", dt=n_dt),
                    ap_dram[:].rearrange("(dt p) j -> p dt j", p=128),
                )
                return [t[:, d * cols:(d + 1) * cols] for d in range(n_dt)]

            wk_sb = load_w(wk, "wk", NDT)
            b_sb = const.tile([128, 14], f32, tag="bias")
            nc.sync.dma_start(b_sb[:], bias[:])
            bq_sb, bk_sb, bv_sb, bo_sb = (
                b_sb[:, 0:2], b_sb[:, 2:4], b_sb[:, 4:6], b_sb[:, 6:14])

            def load_x(xT_dram, tb, split=1):
                xt = inpool.tile([128, NDT * TB], bf16, tag="in")
                hd = NDT // split
                for h in range(split):
                    nc.sync.dma_start(
                        xt[:, h * hd * TB:(h + 1) * hd * TB].rearrange(
                            "p (dt t) -> p dt t", dt=hd),
                        xT_dram[h * hd * 128:(h + 1) * hd * 128,
                                tb * TB:(tb + 1) * TB].rearrange(
                            "(dt p) t -> p dt t", p=128),
                    )
                return xt

            ident = const.tile([128, 128], bf16, tag="ident")
            make_identity(nc, ident[:])

            # ---- persistent activations (feature-major) ----
            qpT = const.tile([128, 2 * S], bf16, tag="qpT")
            kpT = const.tile([128, 2 * S], bf16, tag="kpT")
            vpT = const.tile([128, 2 * S], bf16, tag="vpT")
            vp = const.tile([128, NTT * 2 * VROW], bf16, tag="vp")
            hoT = const.tile([128, 2 * S], bf16, tag="hoT")

            ones_src = const.tile([128, 1], f32, tag="ones_src")
            nc.gpsimd.memset(ones_src[:], 1.0)
            vp_ones = vp[:].rearrange(
                "p (tt seg c) -> p (tt seg) c", tt=NTT, seg=4, c=DK + 1
            )[:, :, DK:DK + 1]
            nc.vector.tensor_copy(vp_ones, ones_src[:].to_broadcast([128, NTT * 4, 1]))

            # ---- projections ----
            def proj_x(xt, w_tiles, b_tile, dstT, tb, pool):
                xtiles = [xt[:, d * TB:(d + 1) * TB] for d in range(NDT)]
                for jt in range(2):
                    ps = pool.tile([128, TB], f32, tag="mm")
                    for d in range(NDT):
                        nc.tensor.matmul(
                            ps[:],
                            lhsT=w_tiles[d][:, jt * 128:(jt + 1) * 128],
                            rhs=xtiles[d],
                            start=(d == 0),
                            stop=(d == NDT - 1),
                        )
                    nc.vector.tensor_scalar_add(
                        dstT[:, jt * S + tb * TB: jt * S + (tb + 1) * TB],
                        ps[:],
                        b_tile[:, jt:jt + 1],
                    )

            def vp_tb(tb, pool):
                for tt in range(tb * 4, (tb + 1) * 4):
                    for jt in range(2):
                        tp = pool.tile([128, 128], bf16, tag="mm")
                        nc.tensor.transpose(
                            tp[:], vpT[:, jt * S + tt * 128: jt * S + (tt + 1) * 128],
                            ident[:],
                        )
                        o = tt * 2 * VROW + jt * VROW
                        nc.vector.tensor_copy(vp[:, o: o + DK], tp[:, 0:DK])
                        nc.vector.tensor_copy(
                            vp[:, o + DK + 1: o + 2 * DK + 1], tp[:, DK:2 * DK])

            # ---- attention pieces ----
            # The PE queue executes in order: a P@V matmul waiting on its exp
            # stalls everything emitted after it. So scores and P@V are split
            # and emitted as a 1-deep software pipeline (next turn's scores
            # always go in front of the previous turn's P@V).
            def turn_scores(sb, jt, tts, pool):
                n = len(tts)
                sc = pool.tile([128, n * 2 * TB], f32, tag="sc")
                for ti, tt in enumerate(tts):
                    for h, p0 in ((0, 0), (1, 64)):
                        nc.tensor.matmul(
                            sc[:, (2 * ti + h) * TB:(2 * ti + h + 1) * TB],
                            lhsT=kpT[p0:p0 + DK, jt * S + tt * 128: jt * S + (tt + 1) * 128],
                            rhs=qpT[p0:p0 + DK, jt * S + sb * TB: jt * S + (sb + 1) * TB],
                        )
                ex = exppool.tile([128, n * 2 * TB], bf16, tag="exp")
                nc.scalar.activation(ex[:], sc[:], AF.Exp, scale=float(1.0 / np.sqrt(DK)))
                return (tts, ex)

            def turn_pv(sb, jt, pend, uA, uB):
                tts, ex = pend
                for ti, tt in enumerate(tts):
                    for h, u in ((0, uA), (1, uB)):
                        o = tt * 2 * VROW + jt * VROW + h * (DK + 1)
                        nc.tensor.matmul(
                            u[:],
                            lhsT=vp[:, o: o + DK + 1],
                            rhs=ex[:, (2 * ti + h) * TB:(2 * ti + h + 1) * TB],
                            start=(tt == 0),
                            stop=(tt == NTT - 1),
                        )

            def norm_pair(sb, jt, uA, uB):
                for h, u in ((0, uA), (1, uB)):
                    usb = usbpool.tile([DK + 1, TB], f32, tag="usb")
                    nc.vector.tensor_copy(usb[:], u[:])
                    rc = nrmpool.tile([1, TB], f32, tag="rc")
                    nc.sync.dma_start(rc[:], usb[DK:DK + 1, :])
                    rc2 = nrmpool.tile([1, TB], f32, tag="rc2")
                    nc.vector.reciprocal_approx_fast(rc2[:], rc[:])
                    rb = nrmpool.tile([DK, TB], f32, tag="rb")
                    nc.gpsimd.partition_broadcast(rb[:], rc2[:])
                    if h == 0:
                        nc.vector.tensor_mul(
                            hoT[0:DK, jt * S + sb * TB: jt * S + (sb + 1) * TB],
                            usb[0:DK, :],
                            rb[:],
                        )
                    else:
                        tmp = nrmpool.tile([DK, TB], bf16, tag="tmp")
                        nc.vector.tensor_mul(tmp[:], usb[0:DK, :], rb[:])
                        nc.sync.dma_start(
                            hoT[DK:2 * DK, jt * S + sb * TB: jt * S + (sb + 1) * TB],
                            tmp[:],
                        )

            # out-proj for query block sb, emitted as 4 [128,1024] psum tiles
            # (2 feature tiles each) interleaved into a pair's sc rotation
            def outproj_group(sb, g, ot, pool, spread_evac=False):
                op = pool.tile([128, 2 * TB], f32, tag="sc")
                for fi in range(2):
                    ft = 2 * g + fi
                    for jt in range(2):
                        nc.tensor.matmul(
                            op[:, fi * TB:(fi + 1) * TB],
                            lhsT=wo_sb[jt][:, ft * 128:(ft + 1) * 128],
                            rhs=hoT[:, jt * S + sb * TB: jt * S + (sb + 1) * TB],
                            start=(jt == 0),
                            stop=(jt == 1),
                        )
                    if spread_evac and fi == 1:
                        nc.scalar.activation(
                            ot[:, ft * TB:(ft + 1) * TB], op[:, fi * TB:(fi + 1) * TB],
                            AF.Identity, bias=bo_sb[:, ft:ft + 1],
                        )
                    else:
                        nc.vector.tensor_scalar_add(
                            ot[:, ft * TB:(ft + 1) * TB], op[:, fi * TB:(fi + 1) * TB],
                            bo_sb[:, ft:ft + 1],
                        )
                if g == 1 or g == 3:
                    h0 = 0 if g == 1 else 512
                    nc.sync.dma_start(
                        out[h0:h0 + 512, sb * TB:(sb + 1) * TB].rearrange(
                            "(ft p) t -> p ft t", p=128),
                        ot[:, (g - 1) * 2 * TB:(g + 1) * 2 * TB].rearrange(
                            "p (ft t) -> p ft t", ft=4),
                    )

            # ================= phase 1: startup =================
            # kv (+all q) projections, interleaved with attention pair (0,0)
            TURN_TTS = [[0, 1], [2], [3, 4], [5], [6, 7], [8],
                        [9, 10], [11], [12, 13], [14], [15]]

            with (
                tc.tile_pool(name="psProj", bufs=2, space="PSUM") as psProj,
                tc.tile_pool(name="psSC0", bufs=2, space="PSUM") as psSC0,
            ):
                uA = psU.tile([DK + 1, TB], f32, tag="U")
                uB = psU.tile([DK + 1, TB], f32, tag="U")
                pend = []
                # DMA order: kT0 | wq | qT0 | wv | vT0  (scores need k+q first)
                xk = load_x(kT, 0, split=2)
                wq_sb = load_w(wq, "wq", NDT)
                xq = load_x(qT, 0)
                wv_sb = load_w(wv, "wv", NDT)
                xv = load_x(vT, 0)
                for tb in range(NTB):
                    if tb > 0:
                        xk = load_x(kT, tb)
                        xv = load_x(vT, tb)
                    proj_x(xk, wk_sb, bk_sb, kpT, tb, psProj)
                    if tb == 0:
                        proj_x(xq, wq_sb, bq_sb, qpT, 0, psProj)
                    proj_x(xv, wv_sb, bv_sb, vpT, tb, psProj)
                    vp_tb(tb, psProj)
                    # emit pair (0, 0) turns whose keys are now projected
                    # (single-tt turns: psSC0 slots are [128, 1024]; lag-2)
                    for tt in range(4 * tb, 4 * tb + 4):
                        pend.append(turn_scores(0, 0, [tt], psSC0))
                        if len(pend) > 2:
                            turn_pv(0, 0, pend.pop(0), uA, uB)
                # q blocks 1-3: fill PE slack under pair (0,0)'s ACT stretch
                for sb in range(1, NTB):
                    xq = load_x(qT, sb)
                    proj_x(xq, wq_sb, bq_sb, qpT, sb, psProj)
                wo_sb = load_w(wo, "wo", 2)
                for p in pend:
                    turn_pv(0, 0, p, uA, uB)
                norm_pair(0, 0, uA, uB)

            # ================= phase 2: attention conveyor =================
            with (
                tc.tile_pool(name="psSCA", bufs=1, space="PSUM") as psSCA,
                tc.tile_pool(name="psSCB", bufs=1, space="PSUM") as psSCB,
            ):
                pairs = [(0, 1)] + [(sb, jt) for sb in range(1, NTB) for jt in range(2)]
                for sb, jt in pairs:
                    uA = psU.tile([DK + 1, TB], f32, tag="U")
                    uB = psU.tile([DK + 1, TB], f32, tag="U")
                    # interleave out-proj of sb-1 into this pair's B-slot turns
                    do_op = (jt == 0 and sb > 0)
                    if do_op:
                        ot = osbpool.tile([128, 8 * TB], bf16, tag="ot")
                    og = 0
                    pend = []
                    for g, tts in enumerate(TURN_TTS):
                        pool = psSCA if len(tts) == 2 else psSCB
                        pend.append(turn_scores(sb, jt, tts, pool))
                        if len(pend) > 2:
                            turn_pv(sb, jt, pend.pop(0), uA, uB)
                        # op groups ride the scB slot between B-turns; placed
                        # after a PV so the preceding exp has drained the slot
                        if do_op and g in (4, 6, 8, 10):
                            outproj_group(sb - 1, og, ot, psSCB)
                            og += 1
                    for p in pend:
                        turn_pv(sb, jt, p, uA, uB)
                    norm_pair(sb, jt, uA, uB)
                # final out-proj (tail: spread evac across scalar+vector)
                ot = osbpool.tile([128, 8 * TB], bf16, tag="ot")
                for g in range(4):
                    outproj_group(NTB - 1, g, ot, psSCB, spread_evac=True)

    nc.compile()
    return nc


def _get_nc():
    global _NC
    if _NC is None:
        _NC = _build()
    return _NC


def make_in_maps(q, k, v, w_q, b_q, w_k, b_k, w_v, b_v, w_o, b_o):
    import ml_dtypes
    cdt = ml_dtypes.bfloat16
    q = np.asarray(q, np.float32)
    k = np.asarray(k, np.float32)
    v = np.asarray(v, np.float32)
    w_q = np.asarray(w_q, np.float32)
    w_k = np.asarray(w_k, np.float32)
    w_v = np.asarray(w_v, np.float32)
    w_o = np.asarray(w_o, np.float32)
    b_q = np.asarray(b_q, np.float32)
    b_k = np.asarray(b_k, np.float32)
    b_v = np.asarray(b_v, np.float32)
    b_o = np.asarray(b_o, np.float32)

    in_maps = []
    for c in range(NCORES):
        b, g = divmod(c, GROUPS)
        js = slice(g * JC, (g + 1) * JC)
        bias2 = lambda x: x[js].reshape(2, 128).T
        bo2 = (b_o.reshape(8, 128).T if g == 0
               else np.zeros((128, 8), np.float32))
        bias_all = np.concatenate(
            [bias2(b_q), bias2(b_k), bias2(b_v), bo2], axis=1)
        in_maps.append({
            "qT": np.ascontiguousarray(q[b].T).astype(cdt),
            "kT": np.ascontiguousarray(k[b].T).astype(cdt),
            "vT": np.ascontiguousarray(v[b].T).astype(cdt),
            "wq": np.ascontiguousarray(w_q[:, js]).astype(cdt),
            "wk": np.ascontiguousarray(w_k[:, js]).astype(cdt),
            "wv": np.ascontiguousarray(w_v[:, js]).astype(cdt),
            "wo": np.ascontiguousarray(w_o[js, :]).astype(cdt),
            "bias": np.ascontiguousarray(bias_all, dtype=np.float32),
        })
    return in_maps


def gather(results):
    out = np.zeros((B, S, D), np.float32)
    for c in range(NCORES):
        b = c // GROUPS
        out[b] += results[c]["out"].T.astype(np.float32)
    return out


def kernel(q, k, v, w_q, b_q, w_k, b_k, w_v, b_v, w_o, b_o, _trace=False):
    from concourse.bass_utils import run_bass_kernel_spmd

    nc = _get_nc()
    in_maps = make_in_maps(q, k, v, w_q, b_q, w_k, b_k, w_v, b_v, w_o, b_o)
    res = run_bass_kernel_spmd(nc, in_maps, core_ids=list(range(NCORES)), trace=_trace)
    out = gather(res.results)
    if _trace:
        kernel.last_exec_time_ns = res.exec_time_ns
        kernel.last_results = res
    return out


# revision 14
# speedup vs baseline: 1.5414x; 1.0916x over previous
"""Multi-head attention (B=2, S=2048, D=1024, H=16) on 8 TRN2 NeuronCores.

Sharding: data-parallel over batch (2 groups of 4 cores), tensor-parallel over
heads within a group (4 heads = 256 feature columns per core). Each core:
  - projects its batch's q/k/v (full D contraction) into its 256-col head slice
  - runs full attention for its 4 heads over the 2048-token sequence
  - applies its 256-row slice of w_o, producing a partial [D, S] output (bf16)
Host sums the 4 partials per batch (+ b_o folded into one core per batch) and
transposes back to [S, D].

V4 structure (vs the 236us baseline):
  - single flat conveyor of per-tt "turns" (scores -> exp -> P@V) across all
    8 (sb, jt) pairs, software-pipelined with lag 2 so the PE queue never
    stalls on an exp wait (the PE is strict in-order; a P@V emitted directly
    after its own scores blocks everything behind it)
  - pair (0,0)'s turns are woven INTO the per-tb projection loop and the
    first projections are per-jt ordered (k-jt0, q-jt0, v-jt0 first), so the
    ScalarE exp conveyor starts at ~17us instead of ~43us
  - DMA emission order = critical path order: wk, bias(one combined tensor),
    kT0, wq, qT0, wv, vT0, then per-tb k/v, then q1-3
  - out-proj and q1-3 projections are woven between turns as PE filler on a
    dedicated 2-bank psOP pool (PSUM: psSC 2x[128,1024] + psOP 2x[128,512]
    + psU 2x[65,512] = 8 banks)
  - `out` is bf16 (halves output DMA traffic; host sums partials in fp32)
Softmax denominator comes from a ones-column appended to each head's V tile
(PSUM-accumulated by the P@V matmul). Scores matmuls are auto row-tiled
(64-partition lhsT at base 0/64 -> PE quadrant concurrency).
"""

import numpy as np

B, S, D, H = 2, 2048, 1024, 16
DK = D // H          # 64
NCORES = 8
GROUPS = 4           # head-groups (cores) per batch
JC = D // GROUPS     # 256 feature columns per core (4 heads)
TB = 512             # token block (matmul moving free dim)
NTB = S // TB        # 4
NDT = D // 128       # 8 contraction tiles for projections
NTT = S // 128       # 16 key-token tiles per sequence
VROW = 2 * (DK + 1)  # 130: per-jt vp row segment (2 heads x (64 v cols + ones))

_NC = None


def _build():
    import concourse.mybir as mybir
    import concourse.tile as tile
    from concourse import bacc
    from concourse.masks import make_identity

    f32 = mybir.dt.float32
    bf16 = mybir.dt.bfloat16
    AF = mybir.ActivationFunctionType

    nc = bacc.Bacc("TRN2", target_bir_lowering=False, debug=False, num_devices=NCORES)

    qT = nc.dram_tensor("qT", [D, S], bf16, kind="ExternalInput").ap()
    kT = nc.dram_tensor("kT", [D, S], bf16, kind="ExternalInput").ap()
    vT = nc.dram_tensor("vT", [D, S], bf16, kind="ExternalInput").ap()
    wq = nc.dram_tensor("wq", [D, JC], bf16, kind="ExternalInput").ap()
    wk = nc.dram_tensor("wk", [D, JC], bf16, kind="ExternalInput").ap()
    wv = nc.dram_tensor("wv", [D, JC], bf16, kind="ExternalInput").ap()
    wo = nc.dram_tensor("wo", [JC, D], bf16, kind="ExternalInput").ap()
    # all biases in one DMA: cols 0-1 bq, 2-3 bk, 4-5 bv, 6-13 bo
    bias = nc.dram_tensor("bias", [128, 14], f32, kind="ExternalInput").ap()
    out = nc.dram_tensor("out", [D, S], bf16, kind="ExternalOutput").ap()

    with tile.TileContext(nc) as tc:
        with (
            tc.tile_pool(name="const", bufs=1) as const,
            tc.tile_pool(name="inp", bufs=5) as inpool,
            tc.tile_pool(name="expp", bufs=4) as exppool,
            tc.tile_pool(name="usb", bufs=4) as usbpool,
            tc.tile_pool(name="nrm", bufs=4) as nrmpool,
            tc.tile_pool(name="osb", bufs=2) as osbpool,
            tc.tile_pool(name="psSC", bufs=2, space="PSUM") as psSC,
            tc.tile_pool(name="psOP", bufs=2, space="PSUM") as psOP,
            tc.tile_pool(name="psU", bufs=2, space="PSUM") as psU,
        ):
            # ---- weights (DMA emission order = critical path order) ----
            def load_w(ap_dram, name, n_dt):
                cols = ap_dram.shape[1]
                t = const.tile([128, n_dt * cols], bf16, tag=name)
                nc.sync.dma_start(
                    t[:].rearrange("p (dt j) -> p dt j", dt=n_dt),
                    ap_dram[:].rearrange("(dt p) j -> p dt j", p=128),
                )
                return [t[:, d * cols:(d + 1) * cols] for d in range(n_dt)]

            wk_sb = load_w(wk, "wk", NDT)
            b_sb = const.tile([128, 14], f32, tag="bias")
            nc.sync.dma_start(b_sb[:], bias[:])
            bq_sb, bk_sb, bv_sb, bo_sb = (
                b_sb[:, 0:2], b_sb[:, 2:4], b_sb[:, 4:6], b_sb[:, 6:14])

            def load_x(xT_dram, tb, split=1):
                xt = inpool.tile([128, NDT * TB], bf16, tag="in")
                hd = NDT // split
                for h in range(split):
                    nc.sync.dma_start(
                        xt[:, h * hd * TB:(h + 1) * hd * TB].rearrange(
                            "p (dt t) -> p dt t", dt=hd),
                        xT_dram[h * hd * 128:(h + 1) * hd * 128,
                                tb * TB:(tb + 1) * TB].rearrange(
                            "(dt p) t -> p dt t", p=128),
                    )
                return xt

            ident = const.tile([128, 128], bf16, tag="ident")
            make_identity(nc, ident[:])

            # ---- persistent activations (feature-major) ----
            qpT = const.tile([128, 2 * S], bf16, tag="qpT")
            kpT = const.tile([128, 2 * S], bf16, tag="kpT")
            vpT = const.tile([128, 2 * S], bf16, tag="vpT")
            vp = const.tile([128, NTT * 2 * VROW], bf16, tag="vp")
            hoT = const.tile([128, 2 * S], bf16, tag="hoT")

            ones_src = const.tile([128, 1], f32, tag="ones_src")
            nc.gpsimd.memset(ones_src[:], 1.0)
            vp_ones = vp[:].rearrange(
                "p (tt seg c) -> p (tt seg) c", tt=NTT, seg=4, c=DK + 1
            )[:, :, DK:DK + 1]
            nc.vector.tensor_copy(vp_ones, ones_src[:].to_broadcast([128, NTT * 4, 1]))

            # ---- projections (one jt half at a time) ----
            def proj_jt(xt, w_tiles, b_tile, dstT, tb, jt):
                ps = psOP.tile([128, TB], f32, tag="mm")
                for d in range(NDT):
                    nc.tensor.matmul(
                        ps[:],
                        lhsT=w_tiles[d][:, jt * 128:(jt + 1) * 128],
                        rhs=xt[:, d * TB:(d + 1) * TB],
                        start=(d == 0),
                        stop=(d == NDT - 1),
                    )
                nc.vector.tensor_scalar_add(
                    dstT[:, jt * S + tb * TB: jt * S + (tb + 1) * TB],
                    ps[:],
                    b_tile[:, jt:jt + 1],
                )

            def vp_jt(tb, jt):
                for tt in range(tb * 4, (tb + 1) * 4):
                    tp = psOP.tile([128, 128], bf16, tag="mm")
                    nc.tensor.transpose(
                        tp[:], vpT[:, jt * S + tt * 128: jt * S + (tt + 1) * 128],
                        ident[:],
                    )
                    o = tt * 2 * VROW + jt * VROW
                    nc.vector.tensor_copy(vp[:, o: o + DK], tp[:, 0:DK])
                    nc.vector.tensor_copy(
                        vp[:, o + DK + 1: o + 2 * DK + 1], tp[:, DK:2 * DK])

            # ---- attention conveyor ----
            # Flat stream of per-tt turns across all (sb, jt) pairs,
            # software-pipelined lag-2: the PE queue is strict in-order, so
            # each turn's P@V (which waits on its exp) is emitted two turns
            # later, with the next scores fills and filler work in between.
            U = {}

            def turn_scores(sb, jt, tt):
                sc = psSC.tile([128, 2 * TB], f32, tag="sc")
                for h, p0 in ((0, 0), (1, 64)):
                    nc.tensor.matmul(
                        sc[:, h * TB:(h + 1) * TB],
                        lhsT=kpT[p0:p0 + DK, jt * S + tt * 128: jt * S + (tt + 1) * 128],
                        rhs=qpT[p0:p0 + DK, jt * S + sb * TB: jt * S + (sb + 1) * TB],
                    )
                ex = exppool.tile([128, 2 * TB], bf16, tag="exp")
                nc.scalar.activation(ex[:], sc[:], AF.Exp, scale=float(1.0 / np.sqrt(DK)))
                return ex

            def norm_pair(sb, jt, uA, uB):
                for h, u in ((0, uA), (1, uB)):
                    usb = usbpool.tile([DK + 1, TB], f32, tag="usb")
                    nc.vector.tensor_copy(usb[:], u[:])
                    rc = nrmpool.tile([1, TB], f32, tag="rc")
                    nc.sync.dma_start(rc[:], usb[DK:DK + 1, :])
                    rc2 = nrmpool.tile([1, TB], f32, tag="rc2")
                    nc.vector.reciprocal_approx_fast(rc2[:], rc[:])
                    rb = nrmpool.tile([DK, TB], f32, tag="rb")
                    nc.gpsimd.partition_broadcast(rb[:], rc2[:])
                    if h == 0:
                        nc.vector.tensor_mul(
                            hoT[0:DK, jt * S + sb * TB: jt * S + (sb + 1) * TB],
                            usb[0:DK, :],
                            rb[:],
                        )
                    else:
                        tmp = nrmpool.tile([DK, TB], bf16, tag="tmp")
                        nc.vector.tensor_mul(tmp[:], usb[0:DK, :], rb[:])
                        nc.sync.dma_start(
                            hoT[DK:2 * DK, jt * S + sb * TB: jt * S + (sb + 1) * TB],
                            tmp[:],
                        )

            pend = []

            def pop_pv():
                sb, jt, tt, ex = pend.pop(0)
                if tt == 0:
                    uA = psU.tile([DK + 1, TB], f32, tag="U", name=f"uA_{sb}_{jt}")
                    uB = psU.tile([DK + 1, TB], f32, tag="U", name=f"uB_{sb}_{jt}")
                    U[(sb, jt)] = (uA, uB)
                uA, uB = U[(sb, jt)]
                for h, u in ((0, uA), (1, uB)):
                    o = tt * 2 * VROW + jt * VROW + h * (DK + 1)
                    nc.tensor.matmul(
                        u[:],
                        lhsT=vp[:, o: o + DK + 1],
                        rhs=ex[:, h * TB:(h + 1) * TB],
                        start=(tt == 0),
                        stop=(tt == NTT - 1),
                    )
                if tt == NTT - 1:
                    norm_pair(sb, jt, uA, uB)

            def push_turn(sb, jt, tt):
                pend.append((sb, jt, tt, turn_scores(sb, jt, tt)))
                if len(pend) > 2:
                    pop_pv()

            # out-proj for query block sb: 8 single-ft groups on psOP
            def outproj_ft(sb, ft, ot, spread_evac=False):
                op = psOP.tile([128, TB], f32, tag="mm")
                for jt in range(2):
                    nc.tensor.matmul(
                        op[:],
                        lhsT=wo_sb[jt][:, ft * 128:(ft + 1) * 128],
                        rhs=hoT[:, jt * S + sb * TB: jt * S + (sb + 1) * TB],
                        start=(jt == 0),
                        stop=(jt == 1),
                    )
                if spread_evac and ft % 2:
                    nc.scalar.activation(
                        ot[:, ft * TB:(ft + 1) * TB], op[:],
                        AF.Identity, bias=bo_sb[:, ft:ft + 1],
                    )
                else:
                    nc.vector.tensor_scalar_add(
                        ot[:, ft * TB:(ft + 1) * TB], op[:], bo_sb[:, ft:ft + 1]
                    )
                if ft == 3 or ft == 7:
                    h0 = 0 if ft == 3 else 512
                    nc.sync.dma_start(
                        out[h0:h0 + 512, sb * TB:(sb + 1) * TB].rearrange(
                            "(ft p) t -> p ft t", p=128),
                        ot[:, (ft - 3) * TB:(ft + 1) * TB].rearrange(
                            "p (ft t) -> p ft t", ft=4),
                    )

            # ================= emission =================
            # startup: tb0 with per-jt ordering so pair (0,0) starts ASAP
            xk = load_x(kT, 0, split=2)
            wq_sb = load_w(wq, "wq", NDT)
            xq = load_x(qT, 0)
            wv_sb = load_w(wv, "wv", NDT)
            xv = load_x(vT, 0)
            proj_jt(xk, wk_sb, bk_sb, kpT, 0, 0)
            proj_jt(xq, wq_sb, bq_sb, qpT, 0, 0)
            proj_jt(xv, wv_sb, bv_sb, vpT, 0, 0)
            vp_jt(0, 0)
            for tt in range(4):
                push_turn(0, 0, tt)
            proj_jt(xk, wk_sb, bk_sb, kpT, 0, 1)
            proj_jt(xq, wq_sb, bq_sb, qpT, 0, 1)
            proj_jt(xv, wv_sb, bv_sb, vpT, 0, 1)
            vp_jt(0, 1)
            for tb in range(1, NTB):
                xk = load_x(kT, tb)
                xv = load_x(vT, tb)
                proj_jt(xk, wk_sb, bk_sb, kpT, tb, 0)
                proj_jt(xv, wv_sb, bv_sb, vpT, tb, 0)
                vp_jt(tb, 0)
                for tt in range(4 * tb, 4 * tb + 2):
                    push_turn(0, 0, tt)
                proj_jt(xk, wk_sb, bk_sb, kpT, tb, 1)
                proj_jt(xv, wv_sb, bv_sb, vpT, tb, 1)
                vp_jt(tb, 1)
                for tt in range(4 * tb + 2, 4 * tb + 4):
                    push_turn(0, 0, tt)
            wo_sb = load_w(wo, "wo", 2)

            # steady conveyor over the remaining 7 pairs, with q1-3
            # projections and out-proj groups woven in as PE filler
            xq_pend = []
            for sb, jt in [(0, 1)] + [(s, j) for s in range(1, NTB) for j in range(2)]:
                do_op = (jt == 0 and sb > 0)
                if do_op:
                    ot = osbpool.tile([128, 8 * TB], bf16, tag="ot")
                if (sb, jt) == (0, 1):
                    for s in range(1, NTB):
                        xq_pend.append((load_x(qT, s), s))
                for tt in range(NTT):
                    push_turn(sb, jt, tt)
                    if (sb, jt) == (0, 1) and tt % 2 == 1 and xq_pend:
                        xqs, s = xq_pend[0]
                        jtq = (tt // 2) % 2
                        proj_jt(xqs, wq_sb, bq_sb, qpT, s, jtq)
                        if jtq == 1:
                            xq_pend.pop(0)
                    if do_op and tt % 2 == 1:
                        outproj_ft(sb - 1, tt // 2, ot)
            while pend:
                pop_pv()
            # final out-proj (tail: spread evac across scalar+vector)
            ot = osbpool.tile([128, 8 * TB], bf16, tag="ot")
            for ft in range(8):
                outproj_ft(NTB - 1, ft, ot, spread_evac=True)

    nc.compile()
    return nc


def _get_nc():
    global _NC
    if _NC is None:
        _NC = _build()
    return _NC


def make_in_maps(q, k, v, w_q, b_q, w_k, b_k, w_v, b_v, w_o, b_o):
    import ml_dtypes
    cdt = ml_dtypes.bfloat16
    q = np.asarray(q, np.float32)
    k = np.asarray(k, np.float32)
    v = np.asarray(v, np.float32)
    w_q = np.asarray(w_q, np.float32)
    w_k = np.asarray(w_k, np.float32)
    w_v = np.asarray(w_v, np.float32)
    w_o = np.asarray(w_o, np.float32)
    b_q = np.asarray(b_q, np.float32)
    b_k = np.asarray(b_k, np.float32)
    b_v = np.asarray(b_v, np.float32)
    b_o = np.asarray(b_o, np.float32)

    in_maps = []
    for c in range(NCORES):
        b, g = divmod(c, GROUPS)
        js = slice(g * JC, (g + 1) * JC)
        bias2 = lambda x: x[js].reshape(2, 128).T
        bo2 = (b_o.reshape(8, 128).T if g == 0
               else np.zeros((128, 8), np.float32))
        bias_all = np.concatenate(
            [bias2(b_q), bias2(b_k), bias2(b_v), bo2], axis=1)
        in_maps.append({
            "qT": np.ascontiguousarray(q[b].T).astype(cdt),
            "kT": np.ascontiguousarray(k[b].T).astype(cdt),
            "vT": np.ascontiguousarray(v[b].T).astype(cdt),
            "wq": np.ascontiguousarray(w_q[:, js]).astype(cdt),
            "wk": np.ascontiguousarray(w_k[:, js]).astype(cdt),
            "wv": np.ascontiguousarray(w_v[:, js]).astype(cdt),
            "wo": np.ascontiguousarray(w_o[js, :]).astype(cdt),
            "bias": np.ascontiguousarray(bias_all, dtype=np.float32),
        })
    return in_maps


def gather(results):
    out = np.zeros((B, S, D), np.float32)
    for c in range(NCORES):
        b = c // GROUPS
        out[b] += results[c]["out"].T.astype(np.float32)
    return out


def kernel(q, k, v, w_q, b_q, w_k, b_k, w_v, b_v, w_o, b_o, _trace=False):
    from concourse.bass_utils import run_bass_kernel_spmd

    nc = _get_nc()
    in_maps = make_in_maps(q, k, v, w_q, b_q, w_k, b_k, w_v, b_v, w_o, b_o)
    res = run_bass_kernel_spmd(nc, in_maps, core_ids=list(range(NCORES)), trace=_trace)
    out = gather(res.results)
    if _trace:
        kernel.last_exec_time_ns = res.exec_time_ns
        kernel.last_results = res
    return out


# revision 18
# speedup vs baseline: 1.5737x; 1.0209x over previous
"""Multi-head attention (B=2, S=2048, D=1024, H=16) on 8 TRN2 NeuronCores.

Sharding: data-parallel over batch (2 groups of 4 cores), tensor-parallel over
heads within a group (4 heads = 256 feature columns per core). Each core:
  - projects its batch's q/k/v (full D contraction) into its 256-col head slice
  - runs full attention for its 4 heads over the 2048-token sequence
  - applies its 256-row slice of w_o, producing a partial [D, S] output (bf16)
Host sums the 4 partials per batch (+ b_o folded into one core per batch) and
transposes back to [S, D].

V4 structure (vs the 236us baseline):
  - single flat conveyor of per-tt "turns" (scores -> exp -> P@V) across all
    8 (sb, jt) pairs, software-pipelined with lag 2 so the PE queue never
    stalls on an exp wait (the PE is strict in-order; a P@V emitted directly
    after its own scores blocks everything behind it)
  - pair (0,0)'s turns are woven INTO the per-tb projection loop and the
    first projections are per-jt ordered (k-jt0, q-jt0, v-jt0 first), so the
    ScalarE exp conveyor starts at ~17us instead of ~43us
  - DMA emission order = critical path order: wk, bias(one combined tensor),
    kT0, wq, qT0, wv, vT0, then per-tb k/v, then q1-3
  - out-proj and q1-3 projections are woven between turns as PE filler on a
    dedicated 2-bank psOP pool (PSUM: psSC 2x[128,1024] + psOP 2x[128,512]
    + psU 2x[65,512] = 8 banks)
  - `out` is bf16 (halves output DMA traffic; host sums partials in fp32)
Softmax denominator comes from a ones-column appended to each head's V tile
(PSUM-accumulated by the P@V matmul). Scores matmuls are auto row-tiled
(64-partition lhsT at base 0/64 -> PE quadrant concurrency).
"""

import numpy as np

B, S, D, H = 2, 2048, 1024, 16
DK = D // H          # 64
NCORES = 8
GROUPS = 4           # head-groups (cores) per batch
JC = D // GROUPS     # 256 feature columns per core (4 heads)
TB = 512             # token block (matmul moving free dim)
NTB = S // TB        # 4
NDT = D // 128       # 8 contraction tiles for projections
NTT = S // 128       # 16 key-token tiles per sequence
VROW = 2 * (DK + 1)  # 130: per-jt vp row segment (2 heads x (64 v cols + ones))

_NC = None


def _build():
    import concourse.mybir as mybir
    import concourse.tile as tile
    from concourse import bacc
    from concourse.masks import make_identity

    f32 = mybir.dt.float32
    bf16 = mybir.dt.bfloat16
    AF = mybir.ActivationFunctionType

    nc = bacc.Bacc("TRN2", target_bir_lowering=False, debug=False, num_devices=NCORES)

    qT = nc.dram_tensor("qT", [D, S], bf16, kind="ExternalInput").ap()
    kT = nc.dram_tensor("kT", [D, S], bf16, kind="ExternalInput").ap()
    vT = nc.dram_tensor("vT", [D, S], bf16, kind="ExternalInput").ap()
    wq = nc.dram_tensor("wq", [D, JC], bf16, kind="ExternalInput").ap()
    wk = nc.dram_tensor("wk", [D, JC], bf16, kind="ExternalInput").ap()
    wv = nc.dram_tensor("wv", [D, JC], bf16, kind="ExternalInput").ap()
    wo = nc.dram_tensor("wo", [JC, D], bf16, kind="ExternalInput").ap()
    # all biases in one DMA: cols 0-1 bq, 2-3 bk, 4-5 bv, 6-13 bo
    bias = nc.dram_tensor("bias", [128, 14], f32, kind="ExternalInput").ap()
    out = nc.dram_tensor("out", [D, S], bf16, kind="ExternalOutput").ap()

    with tile.TileContext(nc) as tc:
        with (
            tc.tile_pool(name="const", bufs=1) as const,
            tc.tile_pool(name="inp", bufs=5) as inpool,
            tc.tile_pool(name="expp", bufs=4) as exppool,
            tc.tile_pool(name="usb", bufs=4) as usbpool,
            tc.tile_pool(name="nrm", bufs=4) as nrmpool,
            tc.tile_pool(name="osb", bufs=2) as osbpool,
            tc.tile_pool(name="psSC", bufs=2, space="PSUM") as psSC,
            tc.tile_pool(name="psOP", bufs=2, space="PSUM") as psOP,
            tc.tile_pool(name="psU", bufs=2, space="PSUM") as psU,
        ):
            # ---- weights (DMA emission order = critical path order) ----
            def load_w(ap_dram, name, n_dt):
                cols = ap_dram.shape[1]
                t = const.tile([128, n_dt * cols], bf16, tag=name)
                nc.sync.dma_start(
                    t[:].rearrange("p (dt j) -> p dt j", dt=n_dt),
                    ap_dram[:].rearrange("(dt p) j -> p dt j", p=128),
                )
                return [t[:, d * cols:(d + 1) * cols] for d in range(n_dt)]

            wk_sb = load_w(wk, "wk", NDT)
            b_sb = const.tile([128, 14], f32, tag="bias")
            nc.sync.dma_start(b_sb[:], bias[:])
            bq_sb, bk_sb, bv_sb, bo_sb = (
                b_sb[:, 0:2], b_sb[:, 2:4], b_sb[:, 4:6], b_sb[:, 6:14])

            def load_x(xT_dram, tb, split=1):
                xt = inpool.tile([128, NDT * TB], bf16, tag="in")
                hd = NDT // split
                for h in range(split):
                    nc.sync.dma_start(
                        xt[:, h * hd * TB:(h + 1) * hd * TB].rearrange(
                            "p (dt t) -> p dt t", dt=hd),
                        xT_dram[h * hd * 128:(h + 1) * hd * 128,
                                tb * TB:(tb + 1) * TB].rearrange(
                            "(dt p) t -> p dt t", p=128),
                    )
                return xt

            ident = const.tile([128, 128], bf16, tag="ident")
            make_identity(nc, ident[:])

            # ---- persistent activations (feature-major) ----
            qpT = const.tile([128, 2 * S], bf16, tag="qpT")
            kpT = const.tile([128, 2 * S], bf16, tag="kpT")
            vpT = const.tile([128, 2 * S], bf16, tag="vpT")
            vp = const.tile([128, NTT * 2 * VROW], bf16, tag="vp")
            hoT = const.tile([128, 2 * S], bf16, tag="hoT")

            ones_src = const.tile([128, 1], f32, tag="ones_src")
            nc.gpsimd.memset(ones_src[:], 1.0)
            vp_ones = vp[:].rearrange(
                "p (tt seg c) -> p (tt seg) c", tt=NTT, seg=4, c=DK + 1
            )[:, :, DK:DK + 1]
            nc.vector.tensor_copy(vp_ones, ones_src[:].to_broadcast([128, NTT * 4, 1]))

            # ---- projections (one jt half at a time) ----
            def proj_jt(xt, w_tiles, b_tile, dstT, tb, jt):
                ps = psOP.tile([128, TB], f32, tag="mm")
                for d in range(NDT):
                    nc.tensor.matmul(
                        ps[:],
                        lhsT=w_tiles[d][:, jt * 128:(jt + 1) * 128],
                        rhs=xt[:, d * TB:(d + 1) * TB],
                        start=(d == 0),
                        stop=(d == NDT - 1),
                    )
                nc.vector.tensor_scalar_add(
                    dstT[:, jt * S + tb * TB: jt * S + (tb + 1) * TB],
                    ps[:],
                    b_tile[:, jt:jt + 1],
                )

            def vp_jt(tb, jt):
                for tt in range(tb * 4, (tb + 1) * 4):
                    tp = psOP.tile([128, 128], bf16, tag="mm")
                    nc.tensor.transpose(
                        tp[:], vpT[:, jt * S + tt * 128: jt * S + (tt + 1) * 128],
                        ident[:],
                    )
                    o = tt * 2 * VROW + jt * VROW
                    nc.vector.tensor_copy(vp[:, o: o + DK], tp[:, 0:DK])
                    nc.vector.tensor_copy(
                        vp[:, o + DK + 1: o + 2 * DK + 1], tp[:, DK:2 * DK])

            # ---- attention conveyor ----
            # Flat stream of per-tt turns across all (sb, jt) pairs,
            # software-pipelined lag-2: the PE queue is strict in-order, so
            # each turn's P@V (which waits on its exp) is emitted two turns
            # later, with the next scores fills and filler work in between.
            U = {}

            def turn_scores(sb, jt, tt):
                sc = psSC.tile([128, 2 * TB], f32, tag="sc")
                for h, p0 in ((0, 0), (1, 64)):
                    nc.tensor.matmul(
                        sc[:, h * TB:(h + 1) * TB],
                        lhsT=kpT[p0:p0 + DK, jt * S + tt * 128: jt * S + (tt + 1) * 128],
                        rhs=qpT[p0:p0 + DK, jt * S + sb * TB: jt * S + (sb + 1) * TB],
                    )
                ex = exppool.tile([128, 2 * TB], bf16, tag="exp")
                nc.scalar.activation(ex[:], sc[:], AF.Exp, scale=float(1.0 / np.sqrt(DK)))
                return ex

            def norm_pair(sb, jt, uA, uB):
                for h, u in ((0, uA), (1, uB)):
                    usb = usbpool.tile([DK + 1, TB], f32, tag="usb")
                    nc.vector.tensor_copy(usb[:], u[:])
                    rc = nrmpool.tile([1, TB], f32, tag="rc")
                    nc.sync.dma_start(rc[:], usb[DK:DK + 1, :])
                    rc2 = nrmpool.tile([1, TB], f32, tag="rc2")
                    nc.vector.reciprocal_approx_fast(rc2[:], rc[:])
                    rb = nrmpool.tile([DK, TB], f32, tag="rb")
                    nc.gpsimd.partition_broadcast(rb[:], rc2[:])
                    if h == 0:
                        nc.vector.tensor_mul(
                            hoT[0:DK, jt * S + sb * TB: jt * S + (sb + 1) * TB],
                            usb[0:DK, :],
                            rb[:],
                        )
                    else:
                        tmp = nrmpool.tile([DK, TB], bf16, tag="tmp")
                        nc.vector.tensor_mul(tmp[:], usb[0:DK, :], rb[:])
                        nc.sync.dma_start(
                            hoT[DK:2 * DK, jt * S + sb * TB: jt * S + (sb + 1) * TB],
                            tmp[:],
                        )

            pend = []

            def pop_pv():
                sb, jt, tt, ex = pend.pop(0)
                if tt == 0:
                    uA = psU.tile([DK + 1, TB], f32, tag="U", name=f"uA_{sb}_{jt}")
                    uB = psU.tile([DK + 1, TB], f32, tag="U", name=f"uB_{sb}_{jt}")
                    U[(sb, jt)] = (uA, uB)
                uA, uB = U[(sb, jt)]
                for h, u in ((0, uA), (1, uB)):
                    o = tt * 2 * VROW + jt * VROW + h * (DK + 1)
                    nc.tensor.matmul(
                        u[:],
                        lhsT=vp[:, o: o + DK + 1],
                        rhs=ex[:, h * TB:(h + 1) * TB],
                        start=(tt == 0),
                        stop=(tt == NTT - 1),
                    )
                if tt == NTT - 1:
                    norm_pair(sb, jt, uA, uB)

            def push_turn(sb, jt, tt):
                pend.append((sb, jt, tt, turn_scores(sb, jt, tt)))
                if len(pend) > 2:
                    pop_pv()

            # out-proj for query block sb: 8 single-ft groups on psOP
            def outproj_ft(sb, ft, ot, spread_evac=False):
                op = psOP.tile([128, TB], f32, tag="mm")
                for jt in range(2):
                    nc.tensor.matmul(
                        op[:],
                        lhsT=wo_sb[jt][:, ft * 128:(ft + 1) * 128],
                        rhs=hoT[:, jt * S + sb * TB: jt * S + (sb + 1) * TB],
                        start=(jt == 0),
                        stop=(jt == 1),
                    )
                if spread_evac and ft % 2:
                    nc.scalar.activation(
                        ot[:, ft * TB:(ft + 1) * TB], op[:],
                        AF.Identity, bias=bo_sb[:, ft:ft + 1],
                    )
                else:
                    nc.vector.tensor_scalar_add(
                        ot[:, ft * TB:(ft + 1) * TB], op[:], bo_sb[:, ft:ft + 1]
                    )
                if ft == 3 or ft == 7:
                    h0 = 0 if ft == 3 else 512
                    nc.sync.dma_start(
                        out[h0:h0 + 512, sb * TB:(sb + 1) * TB].rearrange(
                            "(ft p) t -> p ft t", p=128),
                        ot[:, (ft - 3) * TB:(ft + 1) * TB].rearrange(
                            "p (ft t) -> p ft t", ft=4),
                    )

            # last block's out-proj is split in two passes so the jt0 half
            # (hoT ready one pair earlier) runs during pair (3,1)'s turns and
            # the tail only pays the jt1 matmuls + add-evacs
            def outproj_p1(sb, ft, ot1):
                op = psOP.tile([128, TB], f32, tag="mm")
                nc.tensor.matmul(
                    op[:],
                    lhsT=wo_sb[0][:, ft * 128:(ft + 1) * 128],
                    rhs=hoT[:, sb * TB:(sb + 1) * TB],
                )
                nc.vector.tensor_scalar_add(
                    ot1[:, ft * TB:(ft + 1) * TB], op[:], bo_sb[:, ft:ft + 1]
                )

            def outproj_p2(sb, ft, ot1, ot):
                op = psOP.tile([128, TB], f32, tag="mm")
                nc.tensor.matmul(
                    op[:],
                    lhsT=wo_sb[1][:, ft * 128:(ft + 1) * 128],
                    rhs=hoT[:, S + sb * TB: S + (sb + 1) * TB],
                )
                nc.vector.tensor_add(
                    ot[:, ft * TB:(ft + 1) * TB], op[:],
                    ot1[:, ft * TB:(ft + 1) * TB],
                )
                if ft == 3 or ft == 7:
                    h0 = 0 if ft == 3 else 512
                    nc.sync.dma_start(
                        out[h0:h0 + 512, sb * TB:(sb + 1) * TB].rearrange(
                            "(ft p) t -> p ft t", p=128),
                        ot[:, (ft - 3) * TB:(ft + 1) * TB].rearrange(
                            "p (ft t) -> p ft t", ft=4),
                    )

            # ================= emission =================
            # startup: tb0 with per-jt ordering so pair (0,0) starts ASAP
            xk = load_x(kT, 0, split=2)
            wq_sb = load_w(wq, "wq", NDT)
            xq = load_x(qT, 0)
            wv_sb = load_w(wv, "wv", NDT)
            xv = load_x(vT, 0)
            proj_jt(xk, wk_sb, bk_sb, kpT, 0, 0)
            proj_jt(xq, wq_sb, bq_sb, qpT, 0, 0)
            proj_jt(xv, wv_sb, bv_sb, vpT, 0, 0)
            vp_jt(0, 0)
            for tt in range(4):
                push_turn(0, 0, tt)
            proj_jt(xk, wk_sb, bk_sb, kpT, 0, 1)
            proj_jt(xq, wq_sb, bq_sb, qpT, 0, 1)
            proj_jt(xv, wv_sb, bv_sb, vpT, 0, 1)
            vp_jt(0, 1)
            for tb in range(1, NTB):
                xk = load_x(kT, tb)
                xv = load_x(vT, tb)
                proj_jt(xk, wk_sb, bk_sb, kpT, tb, 0)
                proj_jt(xv, wv_sb, bv_sb, vpT, tb, 0)
                vp_jt(tb, 0)
                for tt in range(4 * tb, 4 * tb + 2):
                    push_turn(0, 0, tt)
                proj_jt(xk, wk_sb, bk_sb, kpT, tb, 1)
                proj_jt(xv, wv_sb, bv_sb, vpT, tb, 1)
                vp_jt(tb, 1)
                for tt in range(4 * tb + 2, 4 * tb + 4):
                    push_turn(0, 0, tt)
            wo_sb = load_w(wo, "wo", 2)

            # steady conveyor over the remaining 7 pairs. PE filler weave:
            #   (s, 1) pairs carry q-block s+1's projection (x2 jt groups)
            #   (s, 0) pairs carry out-proj of block s-1 (x8 ft groups)
            #   (3, 1) also carries out-proj pass 1 of block 3 (x8 groups)
            ot1 = const.tile([128, 8 * TB], f32, tag="ot1")
            for sb, jt in [(0, 1)] + [(s, j) for s in range(1, NTB) for j in range(2)]:
                do_op = (jt == 0 and sb > 0)
                do_q = (jt == 1 and sb < NTB - 1)
                do_p1 = (sb, jt) == (NTB - 1, 1)
                if do_op:
                    ot = osbpool.tile([128, 8 * TB], bf16, tag="ot")
                if do_q:
                    xqs = load_x(qT, sb + 1)
                for tt in range(NTT):
                    push_turn(sb, jt, tt)
                    if do_q and tt in (5, 9):
                        proj_jt(xqs, wq_sb, bq_sb, qpT, sb + 1, (tt - 5) // 4)
                    if do_op and tt % 2 == 1:
                        outproj_ft(sb - 1, tt // 2, ot)
                    if do_p1 and tt % 2 == 1:
                        outproj_p1(NTB - 1, tt // 2, ot1)
            while pend:
                pop_pv()
            # final out-proj pass 2: jt1 matmuls + add-evacs only
            ot = osbpool.tile([128, 8 * TB], bf16, tag="ot")
            for ft in range(8):
                outproj_p2(NTB - 1, ft, ot1, ot)

    nc.compile()
    return nc


def _get_nc():
    global _NC
    if _NC is None:
        _NC = _build()
    return _NC


def make_in_maps(q, k, v, w_q, b_q, w_k, b_k, w_v, b_v, w_o, b_o):
    import ml_dtypes
    cdt = ml_dtypes.bfloat16
    q = np.asarray(q, np.float32)
    k = np.asarray(k, np.float32)
    v = np.asarray(v, np.float32)
    w_q = np.asarray(w_q, np.float32)
    w_k = np.asarray(w_k, np.float32)
    w_v = np.asarray(w_v, np.float32)
    w_o = np.asarray(w_o, np.float32)
    b_q = np.asarray(b_q, np.float32)
    b_k = np.asarray(b_k, np.float32)
    b_v = np.asarray(b_v, np.float32)
    b_o = np.asarray(b_o, np.float32)

    in_maps = []
    for c in range(NCORES):
        b, g = divmod(c, GROUPS)
        js = slice(g * JC, (g + 1) * JC)
        bias2 = lambda x: x[js].reshape(2, 128).T
        bo2 = (b_o.reshape(8, 128).T if g == 0
               else np.zeros((128, 8), np.float32))
        bias_all = np.concatenate(
            [bias2(b_q), bias2(b_k), bias2(b_v), bo2], axis=1)
        in_maps.append({
            "qT": np.ascontiguousarray(q[b].T).astype(cdt),
            "kT": np.ascontiguousarray(k[b].T).astype(cdt),
            "vT": np.ascontiguousarray(v[b].T).astype(cdt),
            "wq": np.ascontiguousarray(w_q[:, js]).astype(cdt),
            "wk": np.ascontiguousarray(w_k[:, js]).astype(cdt),
            "wv": np.ascontiguousarray(w_v[:, js]).astype(cdt),
            "wo": np.ascontiguousarray(w_o[js, :]).astype(cdt),
            "bias": np.ascontiguousarray(bias_all, dtype=np.float32),
        })
    return in_maps


def gather(results):
    out = np.zeros((B, S, D), np.float32)
    for c in range(NCORES):
        b = c // GROUPS
        out[b] += results[c]["out"].T.astype(np.float32)
    return out


def kernel(q, k, v, w_q, b_q, w_k, b_k, w_v, b_v, w_o, b_o, _trace=False):
    from concourse.bass_utils import run_bass_kernel_spmd

    nc = _get_nc()
    in_maps = make_in_maps(q, k, v, w_q, b_q, w_k, b_k, w_v, b_v, w_o, b_o)
    res = run_bass_kernel_spmd(nc, in_maps, core_ids=list(range(NCORES)), trace=_trace)
    out = gather(res.results)
    if _trace:
        kernel.last_exec_time_ns = res.exec_time_ns
        kernel.last_results = res
    return out


# revision 22
# speedup vs baseline: 1.5922x; 1.0118x over previous
"""Multi-head attention (B=2, S=2048, D=1024, H=16) on 8 TRN2 NeuronCores.

Sharding: data-parallel over batch (2 groups of 4 cores), tensor-parallel over
heads within a group (4 heads = 256 feature columns per core). Each core:
  - projects its batch's q/k/v (full D contraction) into its 256-col head slice
  - runs full attention for its 4 heads over the 2048-token sequence
  - applies its 256-row slice of w_o, producing a partial [D, S] output (bf16)
Host sums the 4 partials per batch (+ b_o folded into one core per batch) and
transposes back to [S, D].

V4 structure (vs the 236us baseline):
  - single flat conveyor of per-tt "turns" (scores -> exp -> P@V) across all
    8 (sb, jt) pairs, software-pipelined with lag 2 so the PE queue never
    stalls on an exp wait (the PE is strict in-order; a P@V emitted directly
    after its own scores blocks everything behind it)
  - pair (0,0)'s turns are woven INTO the per-tb projection loop and the
    first projections are per-jt ordered (k-jt0, q-jt0, v-jt0 first), so the
    ScalarE exp conveyor starts at ~17us instead of ~43us
  - DMA emission order = critical path order: wk, bias(one combined tensor),
    kT0, wq, qT0, wv, vT0, then per-tb k/v, then q1-3
  - out-proj and q1-3 projections are woven between turns as PE filler on a
    dedicated 2-bank psOP pool (PSUM: psSC 2x[128,1024] + psOP 2x[128,512]
    + psU 2x[65,512] = 8 banks)
  - `out` is bf16 (halves output DMA traffic; host sums partials in fp32)
Softmax denominator comes from a ones-column appended to each head's V tile
(PSUM-accumulated by the P@V matmul). Scores matmuls are auto row-tiled
(64-partition lhsT at base 0/64 -> PE quadrant concurrency).
"""

import numpy as np

B, S, D, H = 2, 2048, 1024, 16
DK = D // H          # 64
NCORES = 8
GROUPS = 4           # head-groups (cores) per batch
JC = D // GROUPS     # 256 feature columns per core (4 heads)
TB = 512             # token block (matmul moving free dim)
NTB = S // TB        # 4
NDT = D // 128       # 8 contraction tiles for projections
NTT = S // 128       # 16 key-token tiles per sequence
VROW = 2 * (DK + 1)  # 130: per-jt vp row segment (2 heads x (64 v cols + ones))

_NC = None


def _build():
    import concourse.mybir as mybir
    import concourse.tile as tile
    from concourse import bacc
    from concourse.masks import make_identity

    f32 = mybir.dt.float32
    bf16 = mybir.dt.bfloat16
    AF = mybir.ActivationFunctionType

    nc = bacc.Bacc("TRN2", target_bir_lowering=False, debug=False, num_devices=NCORES)

    qT = nc.dram_tensor("qT", [D, S], bf16, kind="ExternalInput").ap()
    kT = nc.dram_tensor("kT", [D, S], bf16, kind="ExternalInput").ap()
    vT = nc.dram_tensor("vT", [D, S], bf16, kind="ExternalInput").ap()
    wq = nc.dram_tensor("wq", [D, JC], bf16, kind="ExternalInput").ap()
    wk = nc.dram_tensor("wk", [D, JC], bf16, kind="ExternalInput").ap()
    wv = nc.dram_tensor("wv", [D, JC], bf16, kind="ExternalInput").ap()
    wo = nc.dram_tensor("wo", [JC, D], bf16, kind="ExternalInput").ap()
    # all biases in one DMA: cols 0-1 bq, 2-3 bk, 4-5 bv, 6-13 bo
    bias = nc.dram_tensor("bias", [128, 14], f32, kind="ExternalInput").ap()
    out = nc.dram_tensor("out", [D, S], bf16, kind="ExternalOutput").ap()

    with tile.TileContext(nc) as tc:
        with (
            tc.tile_pool(name="const", bufs=1) as const,
            tc.tile_pool(name="inp", bufs=5) as inpool,
            tc.tile_pool(name="expp", bufs=4) as exppool,
            tc.tile_pool(name="usb", bufs=4) as usbpool,
            tc.tile_pool(name="nrm", bufs=4) as nrmpool,
            tc.tile_pool(name="osb", bufs=2) as osbpool,
            tc.tile_pool(name="psSC", bufs=2, space="PSUM") as psSC,
            tc.tile_pool(name="psOP", bufs=2, space="PSUM") as psOP,
            tc.tile_pool(name="psU", bufs=2, space="PSUM") as psU,
        ):
            # ---- weights (DMA emission order = critical path order) ----
            def load_w(ap_dram, name, n_dt):
                cols = ap_dram.shape[1]
                t = const.tile([128, n_dt * cols], bf16, tag=name)
                nc.sync.dma_start(
                    t[:].rearrange("p (dt j) -> p dt j", dt=n_dt),
                    ap_dram[:].rearrange("(dt p) j -> p dt j", p=128),
                )
                return [t[:, d * cols:(d + 1) * cols] for d in range(n_dt)]

            wk_sb = load_w(wk, "wk", NDT)
            b_sb = const.tile([128, 14], f32, tag="bias")
            nc.sync.dma_start(b_sb[:], bias[:])
            bq_sb, bk_sb, bv_sb, bo_sb = (
                b_sb[:, 0:2], b_sb[:, 2:4], b_sb[:, 4:6], b_sb[:, 6:14])

            def load_x(xT_dram, tb, split=1):
                xt = inpool.tile([128, NDT * TB], bf16, tag="in")
                hd = NDT // split
                for h in range(split):
                    nc.sync.dma_start(
                        xt[:, h * hd * TB:(h + 1) * hd * TB].rearrange(
                            "p (dt t) -> p dt t", dt=hd),
                        xT_dram[h * hd * 128:(h + 1) * hd * 128,
                                tb * TB:(tb + 1) * TB].rearrange(
                            "(dt p) t -> p dt t", p=128),
                    )
                return xt

            ident = const.tile([128, 128], bf16, tag="ident")
            make_identity(nc, ident[:])

            # HAM warm-up: the PE clock sits at 1.2 GHz until ~3.4us of
            # sustained activity. The input DMAs leave the PE idle for the
            # first ~8us, so burn that window on dummy transposes to hit the
            # first real matmuls at 2.4 GHz.
            _warm_n = [0]

            def warmup(n):
                for _ in range(n):
                    warm = psSC.tile(
                        [128, 128], bf16, tag="sc", name=f"warm{_warm_n[0]}")
                    _warm_n[0] += 1
                    nc.tensor.transpose(warm[:], ident[:], ident[:])

            # ---- persistent activations (feature-major) ----
            qpT = const.tile([128, 2 * S], bf16, tag="qpT")
            kpT = const.tile([128, 2 * S], bf16, tag="kpT")
            vpT = const.tile([128, 2 * S], bf16, tag="vpT")
            vp = const.tile([128, NTT * 2 * VROW], bf16, tag="vp")
            hoT = const.tile([128, 2 * S], bf16, tag="hoT")

            ones_src = const.tile([128, 1], f32, tag="ones_src")
            nc.gpsimd.memset(ones_src[:], 1.0)
            vp_ones = vp[:].rearrange(
                "p (tt seg c) -> p (tt seg) c", tt=NTT, seg=4, c=DK + 1
            )[:, :, DK:DK + 1]
            nc.vector.tensor_copy(vp_ones, ones_src[:].to_broadcast([128, NTT * 4, 1]))

            # ---- projections (one jt half at a time) ----
            def proj_jt(xt, w_tiles, b_tile, dstT, tb, jt):
                ps = psOP.tile([128, TB], f32, tag="mm")
                for d in range(NDT):
                    nc.tensor.matmul(
                        ps[:],
                        lhsT=w_tiles[d][:, jt * 128:(jt + 1) * 128],
                        rhs=xt[:, d * TB:(d + 1) * TB],
                        start=(d == 0),
                        stop=(d == NDT - 1),
                    )
                nc.vector.tensor_scalar_add(
                    dstT[:, jt * S + tb * TB: jt * S + (tb + 1) * TB],
                    ps[:],
                    b_tile[:, jt:jt + 1],
                )

            def vp_jt(tb, jt):
                for tt in range(tb * 4, (tb + 1) * 4):
                    tp = psOP.tile([128, 128], bf16, tag="mm")
                    nc.tensor.transpose(
                        tp[:], vpT[:, jt * S + tt * 128: jt * S + (tt + 1) * 128],
                        ident[:],
                    )
                    o = tt * 2 * VROW + jt * VROW
                    nc.vector.tensor_copy(vp[:, o: o + DK], tp[:, 0:DK])
                    nc.vector.tensor_copy(
                        vp[:, o + DK + 1: o + 2 * DK + 1], tp[:, DK:2 * DK])

            # ---- attention conveyor ----
            # Flat stream of per-tt turns across all (sb, jt) pairs,
            # software-pipelined lag-2: the PE queue is strict in-order, so
            # each turn's P@V (which waits on its exp) is emitted two turns
            # later, with the next scores fills and filler work in between.
            U = {}

            def turn_scores(sb, jt, tt):
                sc = psSC.tile([128, 2 * TB], f32, tag="sc")
                for h, p0 in ((0, 0), (1, 64)):
                    nc.tensor.matmul(
                        sc[:, h * TB:(h + 1) * TB],
                        lhsT=kpT[p0:p0 + DK, jt * S + tt * 128: jt * S + (tt + 1) * 128],
                        rhs=qpT[p0:p0 + DK, jt * S + sb * TB: jt * S + (sb + 1) * TB],
                    )
                ex = exppool.tile([128, 2 * TB], bf16, tag="exp")
                nc.scalar.activation(ex[:], sc[:], AF.Exp, scale=float(1.0 / np.sqrt(DK)))
                return ex

            def norm_pair(sb, jt, uA, uB):
                for h, u in ((0, uA), (1, uB)):
                    usb = usbpool.tile([DK + 1, TB], f32, tag="usb")
                    nc.vector.tensor_copy(usb[:], u[:])
                    rc = nrmpool.tile([1, TB], f32, tag="rc")
                    nc.sync.dma_start(rc[:], usb[DK:DK + 1, :])
                    rc2 = nrmpool.tile([1, TB], f32, tag="rc2")
                    nc.vector.reciprocal_approx_fast(rc2[:], rc[:])
                    rb = nrmpool.tile([DK, TB], f32, tag="rb")
                    nc.gpsimd.partition_broadcast(rb[:], rc2[:])
                    if h == 0:
                        nc.vector.tensor_mul(
                            hoT[0:DK, jt * S + sb * TB: jt * S + (sb + 1) * TB],
                            usb[0:DK, :],
                            rb[:],
                        )
                    else:
                        tmp = nrmpool.tile([DK, TB], bf16, tag="tmp")
                        nc.vector.tensor_mul(tmp[:], usb[0:DK, :], rb[:])
                        nc.sync.dma_start(
                            hoT[DK:2 * DK, jt * S + sb * TB: jt * S + (sb + 1) * TB],
                            tmp[:],
                        )

            pend = []

            def pop_pv():
                sb, jt, tt, ex = pend.pop(0)
                if tt == 0:
                    uA = psU.tile([DK + 1, TB], f32, tag="U", name=f"uA_{sb}_{jt}")
                    uB = psU.tile([DK + 1, TB], f32, tag="U", name=f"uB_{sb}_{jt}")
                    U[(sb, jt)] = (uA, uB)
                uA, uB = U[(sb, jt)]
                for h, u in ((0, uA), (1, uB)):
                    o = tt * 2 * VROW + jt * VROW + h * (DK + 1)
                    nc.tensor.matmul(
                        u[:],
                        lhsT=vp[:, o: o + DK + 1],
                        rhs=ex[:, h * TB:(h + 1) * TB],
                        start=(tt == 0),
                        stop=(tt == NTT - 1),
                    )
                if tt == NTT - 1:
                    norm_pair(sb, jt, uA, uB)

            def push_turn(sb, jt, tt):
                pend.append((sb, jt, tt, turn_scores(sb, jt, tt)))
                if len(pend) > 2:
                    pop_pv()

            # out-proj for query block sb: 8 single-ft groups on psOP
            def outproj_ft(sb, ft, ot, spread_evac=False):
                op = psOP.tile([128, TB], f32, tag="mm")
                for jt in range(2):
                    nc.tensor.matmul(
                        op[:],
                        lhsT=wo_sb[jt][:, ft * 128:(ft + 1) * 128],
                        rhs=hoT[:, jt * S + sb * TB: jt * S + (sb + 1) * TB],
                        start=(jt == 0),
                        stop=(jt == 1),
                    )
                if spread_evac and ft % 2:
                    nc.scalar.activation(
                        ot[:, ft * TB:(ft + 1) * TB], op[:],
                        AF.Identity, bias=bo_sb[:, ft:ft + 1],
                    )
                else:
                    nc.vector.tensor_scalar_add(
                        ot[:, ft * TB:(ft + 1) * TB], op[:], bo_sb[:, ft:ft + 1]
                    )
                if ft == 3 or ft == 7:
                    h0 = 0 if ft == 3 else 512
                    nc.sync.dma_start(
                        out[h0:h0 + 512, sb * TB:(sb + 1) * TB].rearrange(
                            "(ft p) t -> p ft t", p=128),
                        ot[:, (ft - 3) * TB:(ft + 1) * TB].rearrange(
                            "p (ft t) -> p ft t", ft=4),
                    )

            # last block's out-proj is split in two passes so the jt0 half
            # (hoT ready one pair earlier) runs during pair (3,1)'s turns and
            # the tail only pays the jt1 matmuls + add-evacs
            def outproj_p1(sb, ft, ot1):
                op = psOP.tile([128, TB], f32, tag="mm")
                nc.tensor.matmul(
                    op[:],
                    lhsT=wo_sb[0][:, ft * 128:(ft + 1) * 128],
                    rhs=hoT[:, sb * TB:(sb + 1) * TB],
                )
                nc.vector.tensor_scalar_add(
                    ot1[:, ft * TB:(ft + 1) * TB], op[:], bo_sb[:, ft:ft + 1]
                )

            def outproj_p2(sb, ft, ot1, ot):
                op = psOP.tile([128, TB], f32, tag="mm")
                nc.tensor.matmul(
                    op[:],
                    lhsT=wo_sb[1][:, ft * 128:(ft + 1) * 128],
                    rhs=hoT[:, S + sb * TB: S + (sb + 1) * TB],
                )
                nc.vector.tensor_add(
                    ot[:, ft * TB:(ft + 1) * TB], op[:],
                    ot1[:, ft * TB:(ft + 1) * TB],
                )
                if ft == 3 or ft == 7:
                    h0 = 0 if ft == 3 else 512
                    nc.sync.dma_start(
                        out[h0:h0 + 512, sb * TB:(sb + 1) * TB].rearrange(
                            "(ft p) t -> p ft t", p=128),
                        ot[:, (ft - 3) * TB:(ft + 1) * TB].rearrange(
                            "p (ft t) -> p ft t", ft=4),
                    )

            # ================= emission =================
            # startup: tb0 with per-jt ordering so pair (0,0) starts ASAP
            xk = load_x(kT, 0, split=4)
            wq_sb = load_w(wq, "wq", NDT)
            xq = load_x(qT, 0)
            wv_sb = load_w(wv, "wv", NDT)
            xv = load_x(vT, 0)
            warmup(28)
            proj_jt(xk, wk_sb, bk_sb, kpT, 0, 0)
            proj_jt(xq, wq_sb, bq_sb, qpT, 0, 0)
            proj_jt(xv, wv_sb, bv_sb, vpT, 0, 0)
            vp_jt(0, 0)
            for tt in range(4):
                push_turn(0, 0, tt)
            proj_jt(xk, wk_sb, bk_sb, kpT, 0, 1)
            proj_jt(xq, wq_sb, bq_sb, qpT, 0, 1)
            proj_jt(xv, wv_sb, bv_sb, vpT, 0, 1)
            vp_jt(0, 1)
            warmup(10)
            for tb in range(1, NTB):
                xk = load_x(kT, tb)
                xv = load_x(vT, tb)
                proj_jt(xk, wk_sb, bk_sb, kpT, tb, 0)
                proj_jt(xv, wv_sb, bv_sb, vpT, tb, 0)
                vp_jt(tb, 0)
                for tt in range(4 * tb, 4 * tb + 2):
                    push_turn(0, 0, tt)
                proj_jt(xk, wk_sb, bk_sb, kpT, tb, 1)
                proj_jt(xv, wv_sb, bv_sb, vpT, tb, 1)
                vp_jt(tb, 1)
                for tt in range(4 * tb + 2, 4 * tb + 4):
                    push_turn(0, 0, tt)
            wo_sb = load_w(wo, "wo", 2)

            # steady conveyor over the remaining 7 pairs. PE filler weave:
            #   (s, 1) pairs carry q-block s+1's projection (x2 jt groups)
            #   (s, 0) pairs carry out-proj of block s-1 (x8 ft groups)
            #   (3, 1) also carries out-proj pass 1 of block 3 (x8 groups)
            ot1 = const.tile([128, 8 * TB], f32, tag="ot1")
            for sb, jt in [(0, 1)] + [(s, j) for s in range(1, NTB) for j in range(2)]:
                do_op = (jt == 0 and sb > 0)
                do_q = (jt == 1 and sb < NTB - 1)
                do_p1 = (sb, jt) == (NTB - 1, 1)
                if do_op:
                    ot = osbpool.tile([128, 8 * TB], bf16, tag="ot")
                if do_q:
                    xqs = load_x(qT, sb + 1)
                for tt in range(NTT):
                    push_turn(sb, jt, tt)
                    if do_q and tt in (5, 9):
                        proj_jt(xqs, wq_sb, bq_sb, qpT, sb + 1, (tt - 5) // 4)
                    if do_op and tt % 2 == 1:
                        outproj_ft(sb - 1, tt // 2, ot)
                    if do_p1 and tt % 2 == 1:
                        outproj_p1(NTB - 1, tt // 2, ot1)
            while pend:
                pop_pv()
            # final out-proj pass 2: jt1 matmuls + add-evacs only
            ot = osbpool.tile([128, 8 * TB], bf16, tag="ot")
            for ft in range(8):
                outproj_p2(NTB - 1, ft, ot1, ot)

    nc.compile()
    return nc


def _get_nc():
    global _NC
    if _NC is None:
        _NC = _build()
    return _NC


def make_in_maps(q, k, v, w_q, b_q, w_k, b_k, w_v, b_v, w_o, b_o):
    import ml_dtypes
    cdt = ml_dtypes.bfloat16
    q = np.asarray(q, np.float32)
    k = np.asarray(k, np.float32)
    v = np.asarray(v, np.float32)
    w_q = np.asarray(w_q, np.float32)
    w_k = np.asarray(w_k, np.float32)
    w_v = np.asarray(w_v, np.float32)
    w_o = np.asarray(w_o, np.float32)
    b_q = np.asarray(b_q, np.float32)
    b_k = np.asarray(b_k, np.float32)
    b_v = np.asarray(b_v, np.float32)
    b_o = np.asarray(b_o, np.float32)

    in_maps = []
    for c in range(NCORES):
        b, g = divmod(c, GROUPS)
        js = slice(g * JC, (g + 1) * JC)
        bias2 = lambda x: x[js].reshape(2, 128).T
        bo2 = (b_o.reshape(8, 128).T if g == 0
               else np.zeros((128, 8), np.float32))
        bias_all = np.concatenate(
            [bias2(b_q), bias2(b_k), bias2(b_v), bo2], axis=1)
        in_maps.append({
            "qT": np.ascontiguousarray(q[b].T).astype(cdt),
            "kT": np.ascontiguousarray(k[b].T).astype(cdt),
            "vT": np.ascontiguousarray(v[b].T).astype(cdt),
            "wq": np.ascontiguousarray(w_q[:, js]).astype(cdt),
            "wk": np.ascontiguousarray(w_k[:, js]).astype(cdt),
            "wv": np.ascontiguousarray(w_v[:, js]).astype(cdt),
            "wo": np.ascontiguousarray(w_o[js, :]).astype(cdt),
            "bias": np.ascontiguousarray(bias_all, dtype=np.float32),
        })
    return in_maps


def gather(results):
    out = np.zeros((B, S, D), np.float32)
    for c in range(NCORES):
        b = c // GROUPS
        out[b] += results[c]["out"].T.astype(np.float32)
    return out


def kernel(q, k, v, w_q, b_q, w_k, b_k, w_v, b_v, w_o, b_o, _trace=False):
    from concourse.bass_utils import run_bass_kernel_spmd

    nc = _get_nc()
    in_maps = make_in_maps(q, k, v, w_q, b_q, w_k, b_k, w_v, b_v, w_o, b_o)
    res = run_bass_kernel_spmd(nc, in_maps, core_ids=list(range(NCORES)), trace=_trace)
    out = gather(res.results)
    if _trace:
        kernel.last_exec_time_ns = res.exec_time_ns
        kernel.last_results = res
    return out
